# revision 1
# baseline (speedup 1.0000x reference)
"""Trainium2 Bass kernel for nn_AttentionBlock (B=2, T=2048, D=1024, H=16,
Dh=64, Ff=4096), SPMD across 8 NeuronCores in one NEFF launch.

Sharding:
  - Phase 1+2 (QKV projection + attention): 2 heads per core over all 4096
    tokens. Phase 3 (out-proj + residual + LayerNorm + MLP): 512 tokens per
    core, re-sharded via an AllToAll of the attention output (fp8, 0.5 MiB).

Numerics (validated end-to-end in numpy against the reference, see
errstudy.py: full-fp8 attention path lands at ~2e-3 max-rel vs the 2e-2
gate; fp8 MLP would cost ~2e-2 so the MLP matmuls stay bf16):
  - The attention-path matmuls run fp8e4m3, mostly in DoubleRow perf mode
    (2 k-tiles per instruction, 0.5 cycles/output-element): QKV projection,
    Q.K^T scores, attn@V, and the output projection.
  - Weights are pre-scaled x32 on the host so w-values sit in e4m3's normal
    range; the unscale folds into the exp scale (1/8192), the V ones-column
    (=32), and the final-residual scalar_tensor_tensor.
  - alibi is added PRE-exp directly into the scores PSUM by a plain fp8
    matmul against a host-built identity of value 240 (alibi pre-scaled by
    8192/240), so exp(scores*1/8192) on ScalarE emits the final attention
    weights as fp8 with no elementwise alibi multiply. (A DoubleRow inject
    crashes the exec unit on real HW at scale, so it stays plain-mode.)
  - Scores DoubleRow uses a zeroed second k-tile slot (cost-free: DR cost is
    output-size only); attn@V pairs adjacent score k-tiles so both DR slots
    carry real data. The DR stationary width must be 32/64/128, so V is
    padded to 128 columns (col 64 = the softmax-denominator ones column).
  - The softmax denominator falls out of the V ones-column in the attn@V
    matmul; 1/denom is broadcast with gpsimd.partition_broadcast.

kernel(**inputs) takes FULL unsharded inputs, returns the FULL output.
"""

import sys

for _p in ("/opt/trn_rl_repo", "/root/.axon_site/_ro/trn_rl_repo"):
    if _p not in sys.path:
        sys.path.insert(0, _p)

import numpy as np
import ml_dtypes

import concourse.bass as bass
import concourse.tile as tile
from concourse import bacc, mybir
from concourse.bass_utils import run_bass_kernel_spmd

BF16 = ml_dtypes.bfloat16
E4M3 = ml_dtypes.float8_e4m3   # TRN float8e4: max 240

B, T, D, H, Dh, FF = 2, 2048, 1024, 16, 64, 4096
NTOK = B * T            # 4096
NCORES = 8
CHUNK = NTOK // NCORES  # 512 tokens per core
HPC = H // NCORES       # 2 heads per core
KT = T // 128           # 16 k-tiles per batch

WS = 32.0               # host weight pre-scale for fp8 range
EXP_SCALE = 1.0 / (WS * WS * 8.0)   # 1/8192: undo q,k x32 and apply 1/sqrt(Dh)
IDENT_V = 240.0                      # e4m3-exact identity magnitude
AL_SCALE = (WS * WS * 8.0) / IDENT_V  # alibi pre-scale so PSUM alibi = 8192*a

F32 = mybir.dt.float32
F32R = mybir.dt.float32r
BF = mybir.dt.bfloat16
E4 = mybir.dt.float8e4
AF = mybir.ActivationFunctionType
DR = mybir.MatmulPerfMode.DoubleRow

_COMPILED = None


def _build(sim1=False):
    nc = bacc.Bacc("TRN2", target_bir_lowering=False, debug=False,
                   num_devices=1 if sim1 else NCORES)

    # ---- kernel I/O (per core) ----
    # x packed [p, pp, s, tok] = x[tok, (pp*2+s)*128 + p], fp8
    x8_io = nc.dram_tensor("x8", [128, 4, 2, NTOK], E4, kind="ExternalInput").ap()
    # qkv weights x32 packed [p, pp, s, m*128+j] = w[row(m,j), (pp*2+s)*128+p]
    wq8_io = nc.dram_tensor("wq8", [128, 4, 2, 384], E4, kind="ExternalInput").ap()
    # alibi*AL_SCALE transposed: al8[hl, k, q] = AL_SCALE * alibi[0, h, q, k]
    al8_io = nc.dram_tensor("al8", [HPC, T, T], E4, kind="ExternalInput").ap()
    # identity of value IDENT_V for the alibi inject matmul
    i240_io = nc.dram_tensor("i240", [128, 128], E4, kind="ExternalInput").ap()
    ident8_io = nc.dram_tensor("ident8", [128, 128], E4, kind="ExternalInput").ap()
    identb_io = nc.dram_tensor("identb", [128, 128], BF, kind="ExternalInput").ap()
    # out-proj weights x32 packed [p, pp, s, dcol] = WS*w_out[dcol, (pp*2+s)*128+p]
    wo8_io = nc.dram_tensor("wo8", [128, 4, 2, D], E4, kind="ExternalInput").ap()
    x_res_io = nc.dram_tensor("x_res", [CHUNK, D], F32, kind="ExternalInput").ap()
    b_outT_io = nc.dram_tensor("b_outT", [1, D], F32, kind="ExternalInput").ap()
    # mlp-in weights, hi/lo split fp8, packed
    # [p, ff, kk, fin] = (32*w_in_eff)[ff*128+fin, kk*128+p] (hi + residual lo)
    w8i_h_io = nc.dram_tensor("w8i_h", [128, 32, 8, 128], E4, kind="ExternalInput").ap()
    w8i_l_io = nc.dram_tensor("w8i_l", [128, 32, 8, 128], E4, kind="ExternalInput").ap()
    b_inT_io = nc.dram_tensor("b_inT", [128, 32], F32, kind="ExternalInput").ap()
    # mlp-out weights, hi/lo split fp8: [ffp, p, s, dcol] =
    # (32*w_mlp_out)[dcol, (ffp*2+s)*128+p]
    w8o2_h_io = nc.dram_tensor("w8o2_h", [16, 128, 2, D], E4, kind="ExternalInput").ap()
    w8o2_l_io = nc.dram_tensor("w8o2_l", [16, 128, 2, D], E4, kind="ExternalInput").ap()
    out_io = nc.dram_tensor("out", [CHUNK, D], F32, kind="ExternalOutput").ap()

    # ---- internal DRAM ----
    cc_send = nc.dram_tensor("cc_send", [D, CHUNK], E4)
    cc_recv = nc.dram_tensor("cc_recv", [D, CHUNK], E4)

    with tile.TileContext(nc) as tc:
        with tc.tile_pool(name="consts", bufs=1) as consts:
            i240 = consts.tile([128, 128], E4, tag="i240")
            nc.sync.dma_start(i240[:], i240_io)
            ident8 = consts.tile([128, 128], E4, tag="ident8")
            nc.sync.dma_start(ident8[:], ident8_io)
            identb = consts.tile([128, 128], BF, tag="identb")
            nc.sync.dma_start(identb[:], identb_io)
            warm_f = consts.tile([128, 512], F32, tag="warm_f")
            nc.vector.memset(warm_f[:], 0.5)
            warm_rhs = consts.tile([128, 512], F32R, tag="warm_rhs")
            nc.vector.tensor_copy(warm_rhs[:], warm_f[:])
            identr = consts.tile([128, 128], F32R, tag="identr")
            nc.vector.tensor_copy(identr[:], identb[:])
            # phase-3 input tiles (loaded mid-attention; consts pool lives
            # for the whole kernel)
            wo8 = consts.tile([128, 4, 2, D], E4, tag="wo8")
            b_in = consts.tile([128, 32], F32, tag="b_in")
            bob = consts.tile([1, D], F32, tag="bob")
            xrs = consts.tile([128, 4, D], F32, tag="xr")

            with tc.tile_pool(name="qkv", bufs=1) as qkv:
                q8s, k8s, v8s = [], [], []
                for b in range(2):
                    q8 = qkv.tile([128, 2, T], E4, tag=f"q8{b}", name=f"q8{b}")
                    k8 = qkv.tile([128, 2, T], E4, tag=f"k8{b}", name=f"k8{b}")
                    # zero-slot for scores DR second k-tile
                    nc.gpsimd.memset(q8[:, 1, :], 0.0)
                    nc.gpsimd.memset(k8[:, 1, :], 0.0)
                    # v8 [p, ktpair, slot, hl, 128]: cols 0-63 v, col 64 = WS
                    # (softmax denominator), cols 65-127 zero (DR stationary
                    # width must be 32/64/128)
                    v8 = qkv.tile([128, 8, 2, 2, 128], E4, tag=f"v8{b}",
                                  name=f"v8{b}")
                    nc.gpsimd.memset(v8[:], 0.0)
                    nc.vector.memset(v8[:, :, :, :, 64:65], WS)
                    q8s.append(q8); k8s.append(k8); v8s.append(v8)
                yn = [[qkv.tile([64, 1024], E4, tag=f"yn{hl}_{i}",
                                name=f"yn{hl}_{i}") for i in range(4)]
                      for hl in range(2)]

                p1ctx = tc.tile_pool(name="p1x", bufs=1)
                p1x = p1ctx.__enter__()
                p1wctx = tc.tile_pool(name="p1w", bufs=1)
                p1w = p1wctx.__enter__()
                p1tctx = tc.tile_pool(name="p1t", bufs=3)
                p1t = p1tctx.__enter__()
                wq8 = p1w.tile([128, 4, 2, 384], E4, tag="wq8")
                nc.sync.dma_start(wq8[:], wq8_io)

                def proj_pass(b, psp, ptp):
                    q8, k8, v8 = q8s[b], k8s[b], v8s[b]
                    with nc.named_scope(f"qkvproj{b}"):
                        xts = p1x.tile([128, 4, 2, 2048], E4, tag="xt",
                                       name=f"xt{b}")
                        for cc4 in range(4):
                            nc.sync.dma_start(
                                xts[:, :, :, cc4 * 512:(cc4 + 1) * 512],
                                x8_io[:, :, :,
                                      b * 2048 + cc4 * 512:
                                      b * 2048 + (cc4 + 1) * 512])
                        for t in range(4):
                            tsl = slice(t * 512, (t + 1) * 512)
                            for m in range(3):   # q, k, v
                                ps = psp.tile([128, 512], F32, tag="proj",
                                              name=f"proj{b}_{t}_{m}")
                                for pp in range(4):
                                    nc.tensor.matmul(
                                        ps[:],
                                        wq8[:, pp, :, m * 128:(m + 1) * 128],
                                        xts[:, pp, :, tsl],
                                        start=(pp == 0), stop=(pp == 3),
                                        perf_mode=DR)
                                # copies on the otherwise-idle Act engine
                                if m == 0:
                                    nc.vector.tensor_copy(q8[:, 0, tsl], ps[:])
                                elif m == 1:
                                    nc.vector.tensor_copy(k8[:, 0, tsl], ps[:])
                                else:
                                    vt = p1t.tile([128, 512], BF, tag="vt",
                                                  name=f"vt{b}_{t}")
                                    nc.scalar.copy(vt[:], ps[:])
                                    for j in range(4):
                                        ti = t * 4 + j
                                        pt = ptp.tile([128, 128], BF,
                                                      tag="pt",
                                                      name=f"pt{b}_{ti}")
                                        nc.tensor.transpose(
                                            pt[:],
                                            vt[:, j * 128:(j + 1) * 128],
                                            identb[:])
                                        nc.vector.tensor_copy(
                                            v8[:, ti // 2, ti % 2, :, 0:64],
                                            pt[:].rearrange(
                                                "p (h d) -> p h d", h=2))

                # batch 0 with wide PSUM pools (closed before attention)
                with tc.tile_pool(name="p1ps", bufs=4, space="PSUM") as p1ps, \
                     tc.tile_pool(name="p1pt", bufs=2, space="PSUM") as p1pt:
                    for wi_ in range(4):
                        wps = p1pt.tile([128, 512], F32, tag="pt",
                                        name=f"warms{wi_}")
                        nc.tensor.matmul(wps[:], identr[:], warm_rhs[:],
                                         start=True, stop=True)
                    proj_pass(0, p1ps, p1pt)

                with nc.named_scope("attn"), \
                     tc.tile_pool(name="alb", bufs=24) as albp, \
                     tc.tile_pool(name="exps", bufs=6) as expp, \
                     tc.tile_pool(name="sps", bufs=2, space="PSUM") as spsp, \
                     tc.tile_pool(name="yups", bufs=1, space="PSUM") as yupp, \
                     tc.tile_pool(name="p2ps", bufs=1, space="PSUM") as p2ps, \
                     tc.tile_pool(name="p2pt", bufs=1, space="PSUM") as p2pt, \
                     tc.tile_pool(name="nrm", bufs=4) as nrmp:
                    al_cache = {}

                    def attn_pass(hl, qc, b):
                        if (hl, qc) not in al_cache:
                            al_cache[(hl, qc)] = [
                                albp.tile([128, 1024], E4, tag="al",
                                          name=f"al{hl}_{qc}_{kt}")
                                for kt in range(KT)]
                        als = al_cache[(hl, qc)]
                        hsl = slice(hl * 64, (hl + 1) * 64)
                        yu = yupp.tile([128, 1024], F32, tag="yu",
                                       name=f"yu{hl}_{qc}_{b}")
                        ex8 = None
                        for kt in range(KT):
                            if b == 0:
                                nc.sync.dma_start(
                                    als[kt][:],
                                    al8_io[hl, kt * 128:(kt + 1) * 128,
                                           qc * 1024:(qc + 1) * 1024])
                            sp = spsp.tile([128, 1024], F32, tag="sp",
                                           name=f"sp{hl}_{qc}_{b}_{kt}")
                            for qh in range(2):
                                qsl = slice(qc * 1024 + qh * 512,
                                            qc * 1024 + (qh + 1) * 512)
                                osl = slice(qh * 512, (qh + 1) * 512)
                                nc.tensor.matmul(
                                    sp[:, osl],
                                    k8s[b][hsl, :, kt * 128:(kt + 1) * 128],
                                    q8s[b][hsl, :, qsl],
                                    start=True, stop=False, perf_mode=DR)
                                nc.tensor.matmul(
                                    sp[:, osl],
                                    i240[:],
                                    als[kt][:, osl],
                                    start=False, stop=True)
                            if kt % 2 == 0:
                                ex8 = expp.tile([128, 2, 1024], E4, tag="ex",
                                                name=f"ex{hl}_{qc}_{b}_{kt}")
                            nc.scalar.activation(ex8[:, kt % 2, :], sp[:],
                                                 AF.Exp, scale=EXP_SCALE)
                            if kt % 2 == 1:
                                ktp = kt // 2
                                for qh in range(2):
                                    osl = slice(qh * 512, (qh + 1) * 512)
                                    nc.tensor.matmul(
                                        yu[:, osl],
                                        v8s[b][:, ktp, :, hl, :],
                                        ex8[:, :, osl],
                                        start=(ktp == 0), stop=(ktp == 7),
                                        perf_mode=DR)
                        rec = nrmp.tile([1, 1024], F32, tag="rec",
                                        name=f"rec{hl}_{qc}_{b}")
                        nc.vector.reciprocal(rec[:], yu[64:65, :])
                        bc = nrmp.tile([64, 1024], F32, tag="bc",
                                       name=f"bc{hl}_{qc}_{b}")
                        nc.gpsimd.partition_broadcast(bc[:], rec[:])
                        i = b * 2 + qc
                        nc.vector.tensor_mul(
                            yn[hl][i][:], yu[0:64, :], bc[:])
                        nc.sync.dma_start(
                            bass.AP(tensor=cc_send,
                                    offset=(2 * i * 128 + hl * 64) * 512,
                                    ap=[[512, 64], [128 * 512, 2], [1, 512]]),
                            yn[hl][i][:].rearrange("p (h c) -> p h c", h=2))

                    # first attention pass overlaps batch-1's projection
                    # (small PSUM pools p2ps/p2pt keep the total <= 8 banks)
                    attn_pass(0, 0, 0)
                    proj_pass(1, p2ps, p2pt)
                    # no-dependency phase-3 loads: issued here so they
                    # transfer during attention, after the critical
                    # first-pass alibi and batch-1 x loads
                    nc.sync.dma_start(wo8[:], wo8_io[:])
                    nc.sync.dma_start(b_in[:], b_inT_io[:])
                    nc.sync.dma_start(bob[:], b_outT_io[:])
                    x_res_r = x_res_io.rearrange("(t p) d -> p t d", p=128)
                    for tt_ in range(4):
                        nc.sync.dma_start(xrs[:, tt_, :], x_res_r[:, tt_, :])
                    for hl in range(2):
                        for qc in range(2):
                            for b in range(2):
                                if (hl, qc, b) != (0, 0, 0):
                                    attn_pass(hl, qc, b)

                for ctx in (p1tctx, p1wctx, p1ctx):
                    ctx.__exit__(None, None, None)

                with nc.named_scope("a2a"):
                    if sim1:
                        nc.sync.dma_start(cc_recv[:], cc_send[:])
                    else:
                        nc.gpsimd.collective_compute(
                            "AllToAll", mybir.AluOpType.bypass,
                            replica_groups=[list(range(NCORES))],
                            ins=[cc_send[:]], outs=[cc_recv[:]])

            # ---------------- phase 3: out-proj + LN + MLP ----------------
            with nc.named_scope("mlp"), \
                 tc.tile_pool(name="p3w", bufs=1) as p3w, \
                 tc.tile_pool(name="p3acc", bufs=2, space="PSUM") as p3acc, \
                 tc.tile_pool(name="p3mo", bufs=4, space="PSUM") as p3mo, \
                 tc.tile_pool(name="p3pt", bufs=2, space="PSUM") as p3pt, \
                 tc.tile_pool(name="p3sb", bufs=1) as p3sb, \
                 tc.tile_pool(name="p3r", bufs=3) as p3r, \
                 tc.tile_pool(name="p3s", bufs=4) as p3s, \
                 tc.tile_pool(name="mlpw", bufs=4) as mlpw:
                for wi_ in range(0):
                    wps = p3pt.tile([128, 512], F32, tag="pt3",
                                    name=f"warm{wi_}")
                    nc.tensor.matmul(wps[:], identr[:], warm_rhs[:],
                                     start=True, stop=True)
                yrT = p3w.tile([128, 8, 512], E4, tag="yrT")
                for pp_ in range(4):
                    nc.scalar.dma_start(
                        yrT[:, 2 * pp_:2 * pp_ + 2, :],
                        bass.AP(tensor=cc_recv, offset=2 * pp_ * 128 * 512,
                                ap=[[512, 128], [128 * 512, 2], [1, 512]]))
                bb = p3sb.tile([128, D], F32, tag="bb")
                nc.gpsimd.partition_broadcast(bb[:], bob[:])

                y_sb = p3sb.tile([128, 4, D], F32, tag="y_sb")
                y2_sb = p3sb.tile([128, 4, D], F32, tag="y2_sb")
                for tt in range(4):
                    xr = xrs[:, tt, :]
                    for dc in range(2):
                        ps = p3acc.tile([128, 512], F32, tag="acc")
                        for pp in range(4):
                            nc.tensor.matmul(
                                ps[:],
                                yrT[:, 2 * pp:2 * pp + 2,
                                    tt * 128:(tt + 1) * 128],
                                wo8[:, pp, :, dc * 512:(dc + 1) * 512],
                                start=(pp == 0), stop=(pp == 3),
                                perf_mode=DR)
                        # psum = WS * (y @ w_out); add residual with unscale
                        nc.vector.scalar_tensor_tensor(
                            y_sb[:, tt, dc * 512:(dc + 1) * 512], ps[:],
                            1.0 / WS,
                            xrs[:, tt, dc * 512:(dc + 1) * 512],
                            mybir.AluOpType.mult, mybir.AluOpType.add)
                        # y2 = y + b_mlp_out on the idle Pool engine
                        nc.gpsimd.tensor_add(
                            y2_sb[:, tt, dc * 512:(dc + 1) * 512],
                            y_sb[:, tt, dc * 512:(dc + 1) * 512],
                            bb[:, dc * 512:(dc + 1) * 512])

                # LayerNorm -> h_norm (bf16) -> transpose -> hT8 + hTlo
                # (hi/lo fp8 split: h ~ hT8 + hTlo to bf16-class accuracy)

                # MLP in + gelu -> hmT (Ff-major bf16).
                # 3-pass hi/lo DR: h8*Wh + h8*Wl + hlo*Wh (12 DR instrs/ff
                # vs 8 bf16 matmuls; the dropped hlo*Wl term is ~1e-3 rel).
                hT8 = p3sb.tile([128, 8, 512], E4, tag="hT8")
                hTlo = p3sb.tile([128, 8, 512], E4, tag="hTlo")
                for tt in range(4):
                    stats = p3s.tile([128, 2, 6], F32, tag="stats")
                    for g in range(2):
                        nc.vector.bn_stats(
                            stats[:, g, :],
                            y_sb[:, tt, g * 512:(g + 1) * 512])
                    mv = p3s.tile([128, 2], F32, tag="mv")
                    nc.vector.bn_aggr(mv[:], stats[:])
                    eps = p3s.tile([128, 1], F32, tag="eps")
                    nc.vector.memset(eps[:], 1e-5)
                    sd = p3s.tile([128, 1], F32, tag="sd")
                    nc.scalar.activation(sd[:], mv[:, 1:2], AF.Sqrt,
                                         bias=eps[:], scale=1.0)
                    rstd = p3s.tile([128, 1], F32, tag="rstd")
                    nc.vector.reciprocal(rstd[:], sd[:])
                    nb2 = p3s.tile([128, 1], F32, tag="nb2")
                    nc.vector.scalar_tensor_tensor(
                        nb2[:], mv[:, 0:1], -1.0, rstd[:],
                        mybir.AluOpType.mult, mybir.AluOpType.mult)
                    hn = p3r.tile([128, D], BF, tag="hn")
                    nc.scalar.activation(hn[:], y_sb[:, tt, :], AF.Identity,
                                         bias=nb2[:], scale=rstd[:])
                    for dc in range(8):
                        pt = p3pt.tile([128, 128], BF, tag="pt3")
                        nc.tensor.transpose(
                            pt[:], hn[:, dc * 128:(dc + 1) * 128], identb[:])
                        tsl = slice(tt * 128, (tt + 1) * 128)
                        nc.vector.tensor_copy(hT8[:, dc, tsl], pt[:])
                        nc.vector.tensor_sub(hTlo[:, dc, tsl], pt[:],
                                             hT8[:, dc, tsl])

                # MLP in + gelu, 3-pass hi/lo DR
                hmT8 = p3sb.tile([128, 32, 512], E4, tag="hmT8")
                hmTlo = p3sb.tile([128, 32, 512], E4, tag="hmTlo")
                for ff in range(32):
                    wih = mlpw.tile([128, 8, 128], E4, tag="wih")
                    nc.sync.dma_start(wih[:], w8i_h_io[:, ff, :, :])
                    wil = mlpw.tile([128, 8, 128], E4, tag="wil")
                    nc.sync.dma_start(wil[:], w8i_l_io[:, ff, :, :])
                    ps = p3acc.tile([128, 512], F32, tag="acc")
                    first, last = (0, 0), (2, 3)
                    for pi, (w, h) in enumerate(
                            ((wih, hT8), (wil, hT8), (wih, hTlo))):
                        for pp in range(4):
                            nc.tensor.matmul(
                                ps[:], w[:, 2 * pp:2 * pp + 2, :],
                                h[:, 2 * pp:2 * pp + 2, :],
                                start=((pi, pp) == first),
                                stop=((pi, pp) == last), perf_mode=DR)
                    t2 = p3r.tile([128, 512], BF, tag="t2",
                                  name=f"t2_{ff}")
                    nc.scalar.activation(t2[:], ps[:], AF.Gelu,
                                         bias=b_in[:, ff:ff + 1],
                                         scale=1.0 / WS)
                    # hi/lo fp8 split of the gelu output for the DR mlp_out
                    nc.vector.tensor_copy(hmT8[:, ff, :], t2[:])
                    nc.gpsimd.tensor_sub(hmTlo[:, ff, :], t2[:],
                                         hmT8[:, ff, :])

                # MLP out + final residual: 3-pass hi/lo DR over ff-pairs
                out_r = out_io.rearrange("(t p) d -> p t d", p=128)
                for dc in range(2):
                    pss = [p3mo.tile([128, 512], F32, tag="mo",
                                     name=f"mo{dc}_{i}") for i in range(4)]
                    for ffp in range(16):
                        w2h = mlpw.tile([128, 2, 512], E4, tag="w2h")
                        nc.sync.dma_start(
                            w2h[:], w8o2_h_io[ffp, :, :,
                                              dc * 512:(dc + 1) * 512])
                        w2l = mlpw.tile([128, 2, 512], E4, tag="w2l")
                        nc.sync.dma_start(
                            w2l[:], w8o2_l_io[ffp, :, :,
                                              dc * 512:(dc + 1) * 512])
                        for tt in range(4):
                            tsl = slice(tt * 128, (tt + 1) * 128)
                            for pi, (g, w) in enumerate(
                                    ((hmT8, w2h), (hmT8, w2l),
                                     (hmTlo, w2h))):
                                nc.tensor.matmul(
                                    pss[tt][:],
                                    g[:, 2 * ffp:2 * ffp + 2, tsl], w[:],
                                    start=(ffp == 0 and pi == 0),
                                    stop=(ffp == 15 and pi == 2),
                                    perf_mode=DR)
                    for tt in range(4):
                        fin = p3s.tile([128, 512], F32, tag="fin")
                        # psum carries x32 from the weight prescale
                        nc.vector.scalar_tensor_tensor(
                            fin[:], pss[tt][:], 1.0 / WS,
                            y2_sb[:, tt, dc * 512:(dc + 1) * 512],
                            mybir.AluOpType.mult, mybir.AluOpType.add)
                        nc.sync.dma_start(
                            out_r[:, tt, dc * 512:(dc + 1) * 512], fin[:])

    nc.compile()
    return nc


def _host_prep(x, alibi, ln1_w, w_qkv, w_out, ln2_w, w_mlp_in, b_mlp_in,
               w_mlp_out, b_mlp_out):
    f32 = np.float32
    x = np.asarray(x, f32)
    x_flat = np.ascontiguousarray(x.reshape(NTOK, D))
    w_qkv = np.asarray(w_qkv, f32)
    w_out = np.asarray(w_out, f32)
    w_mlp_in = np.asarray(w_mlp_in, f32)
    w_mlp_out = np.asarray(w_mlp_out, f32)
    b_mlp_in = np.asarray(b_mlp_in, f32)
    b_mlp_out = np.asarray(b_mlp_out, f32)
    ln2_w = np.asarray(ln2_w, f32)
    alibi = np.asarray(alibi, f32)

    # x8 [p, pp, s, tok] = x[tok, (pp*2+s)*128 + p]
    x8 = np.ascontiguousarray(
        x_flat.T.reshape(4, 2, 128, NTOK).transpose(2, 0, 1, 3)).astype(E4M3)

    # identities
    i240 = (IDENT_V * np.eye(128, dtype=f32)).astype(E4M3)
    ident8 = np.eye(128, dtype=f32).astype(E4M3)
    identb = np.eye(128, dtype=f32).astype(BF16)

    # out-proj weights: wo8[p, pp, s, dcol] = WS * w_out[dcol, (pp*2+s)*128+p]
    wo8 = np.ascontiguousarray(
        (WS * w_out).T.reshape(4, 2, 128, D).transpose(2, 0, 1, 3)
    ).astype(E4M3)

    w_in_eff = WS * w_mlp_in * ln2_w[None, :]     # (FF, D), x32 for fp8
    w_inP = np.ascontiguousarray(
        w_in_eff.reshape(32, 128, 8, 128).transpose(3, 0, 2, 1))
    w8i_h = w_inP.astype(E4M3)
    w8i_l = (w_inP - w8i_h.astype(np.float32)).astype(E4M3)
    # w2P [ffp, p, s, dcol] = (32*w_mlp_out)[dcol, (ffp*2+s)*128+p]
    w2P = np.ascontiguousarray(
        (WS * w_mlp_out).T.reshape(16, 2, 128, D).transpose(0, 2, 1, 3))
    w8o2_h = w2P.astype(E4M3)
    w8o2_l = (w2P - w8o2_h.astype(np.float32)).astype(E4M3)
    b_inT = np.ascontiguousarray(b_mlp_in.reshape(32, 128).T)

    in_maps = []
    for c in range(NCORES):
        h0 = HPC * c
        qrows = WS * w_qkv[h0 * Dh:(h0 + HPC) * Dh]
        krows = WS * w_qkv[H * Dh + h0 * Dh:H * Dh + (h0 + HPC) * Dh]
        vrows = WS * w_qkv[2 * H * Dh + h0 * Dh:2 * H * Dh + (h0 + HPC) * Dh]
        wq = np.concatenate([qrows, krows, vrows], 0)   # [384, D]
        # wq8 [p, pp, s, mcol] = wq[mcol, (pp*2+s)*128 + p]
        wq8 = np.ascontiguousarray(
            wq.T.reshape(4, 2, 128, 384).transpose(2, 0, 1, 3)).astype(E4M3)
        # al8 [hl, k, q] = AL_SCALE * alibi[0, h, q, k]
        alc = AL_SCALE * alibi[0, h0:h0 + HPC]          # [HPC, T(q), T(k)]
        al8 = np.ascontiguousarray(alc.transpose(0, 2, 1)).astype(E4M3)
        x_res = np.ascontiguousarray(x_flat[c * CHUNK:(c + 1) * CHUNK])
        in_maps.append({
            "x8": x8, "wq8": wq8, "al8": al8, "i240": i240,
            "ident8": ident8, "identb": identb, "wo8": wo8,
            "x_res": x_res, "b_outT": b_mlp_out.reshape(1, D),
            "w8i_h": w8i_h, "w8i_l": w8i_l,
            "b_inT": b_inT, "w8o2_h": w8o2_h, "w8o2_l": w8o2_l,
        })
    return in_maps


def _get_compiled():
    global _COMPILED
    if _COMPILED is None:
        _COMPILED = _build()
    return _COMPILED


def kernel(_trace=False, **inputs):
    nc = _get_compiled()
    in_maps = _host_prep(**inputs)
    res = None
    for attempt in range(3):
        try:
            res = run_bass_kernel_spmd(nc, in_maps,
                                       core_ids=list(range(NCORES)),
                                       trace=_trace)
            break
        except Exception:
            if attempt == 2:
                raise
    out = np.concatenate([res.results[c]["out"] for c in range(NCORES)], 0)
    out = out.reshape(B, T, D).astype(np.float32)
    if _trace:
        return out, res
    return out



# revision 57
# speedup vs baseline: 1.1977x; 1.1977x over previous
"""Trainium2 Bass kernel for nn_AttentionBlock (B=2, T=2048, D=1024, H=16,
Dh=64, Ff=4096), SPMD across 8 NeuronCores in one NEFF launch.

Sharding:
  - Phase 1+2 (QKV projection + attention): 2 heads per core over all 4096
    tokens. Phase 3 (out-proj + residual + LayerNorm + MLP): 512 tokens per
    core, re-sharded via an AllToAll of the attention output (fp8, 0.5 MiB).

Numerics (validated end-to-end in numpy against the reference, see
errstudy2.py: ~2e-3 max-rel vs the 2e-2 gate):
  - Attention matmuls run fp8e4m3 DoubleRow as before (QKV projection,
    Q.K^T scores, attn@V, output projection). Weights pre-scaled x32.
  - The softmax exp is split across two engines per k-tile (the Act engine
    was the attention bottleneck at ~1us per [128,1024] exp):
      * kt < NAK: alibi injected into the scores PSUM by a plain fp8 matmul
        against a 240-valued identity (as before), then Act exp ->
        fp8 attention weights.
      * kt >= NAK: DVE computes the fp8 weights directly via the
        exp bit-trick: i8 = psum*(8*log2e/8192) + (alibi*8*log2e + 56 [+rnd])
        written as int8 and bit-viewed as e4m3 ~= exp(s+alibi). The f16
        alibi-bias tile fuses the alibi add, so no PE inject is needed.
    The bit-trick's mantissa-interpolation error (~3%) is the same class as
    the e4m3 quantization the Act path already incurs; softmax averaging
    washes both out (errstudy2: 2.01e-3 vs 2.13e-3).
  - Scores DoubleRow uses a zeroed second k-tile slot; attn@V pairs adjacent
    score k-tiles. V padded to 128 cols (col 64 = softmax-denominator ones
    column = 32).
  - mlp_in runs 2-pass (h8 @ (W_hi + W_lo): weights at bf16-class accuracy,
    activations fp8); mlp_out stays 3-pass hi/lo. End-to-end rel_absmax
    measured 1.204e-2 on HW (errstudy2 V4 predicted 1.206e-2) vs the 2e-2
    gate. The A2A is split by head-half so the first collective overlaps
    the second half of attention, and the out-projection runs in two
    partition-half stages with the residual injected via f32r identity
    matmuls (readouts become pure Act/DVE scaled copies).

kernel(**inputs) takes FULL unsharded inputs, returns the FULL output.
"""

import sys

for _p in ("/opt/trn_rl_repo", "/root/.axon_site/_ro/trn_rl_repo"):
    if _p not in sys.path:
        sys.path.insert(0, _p)

import numpy as np
import ml_dtypes

import concourse.bass as bass
import concourse.tile as tile
from concourse import bacc, mybir
from concourse.bass_utils import run_bass_kernel_spmd

BF16 = ml_dtypes.bfloat16
E4M3 = ml_dtypes.float8_e4m3   # TRN float8e4: max 240

B, T, D, H, Dh, FF = 2, 2048, 1024, 16, 64, 4096
NTOK = B * T            # 4096
NCORES = 8
CHUNK = NTOK // NCORES  # 512 tokens per core
HPC = H // NCORES       # 2 heads per core
KT = T // 128           # 16 k-tiles per batch

# Interleaved engine assignment of the 16 k-tiles per pass: Act tiles get
# true exp (+PE alibi inject), DVE tiles get the fused bit-trick exp.
# Interleaving keeps both engines fed as the PE emits scores in kt order.
ACT_KTS = (0, 1, 3, 4, 6, 8, 10, 12, 14, 15)
DVE_KTS = (2, 5, 7, 9, 11, 13)
NAK = len(ACT_KTS)
NBK = len(DVE_KTS)
A_POS = {kt: i for i, kt in enumerate(ACT_KTS)}
D_POS = {kt: i for i, kt in enumerate(DVE_KTS)}

WS = 32.0               # host weight pre-scale for fp8 range
EXP_SCALE = 1.0 / (WS * WS * 8.0)   # 1/8192: undo q,k x32 and apply 1/sqrt(Dh)
IDENT_V = 240.0                      # e4m3-exact identity magnitude
AL_SCALE = (WS * WS * 8.0) / IDENT_V  # alibi pre-scale so PSUM alibi = 8192*a
LOG2E8 = float(8.0 * np.log2(np.e))  # e4m3 bit-trick exp slope
BT_SCALE = LOG2E8 / 8192.0
BT_BIAS = 56.0          # e4m3 exponent-bias term (see _host_prep; rounding
                        # offset calibrated on HW via probe.py)

F32 = mybir.dt.float32
F32R = mybir.dt.float32r
BF = mybir.dt.bfloat16
E4 = mybir.dt.float8e4
F16 = mybir.dt.float16
I8 = mybir.dt.int8
AF = mybir.ActivationFunctionType
DR = mybir.MatmulPerfMode.DoubleRow
MUL = mybir.AluOpType.mult
ADD = mybir.AluOpType.add

_COMPILED = None


def _build(sim1=False):
    nc = bacc.Bacc("TRN2", target_bir_lowering=False, debug=False,
                   num_devices=1 if sim1 else NCORES)

    # ---- kernel I/O (per core) ----
    # x packed [p, pp, s, tok] = x[tok, (pp*2+s)*128 + p], fp8
    x8_io = nc.dram_tensor("x8", [128, 4, 2, NTOK], E4, kind="ExternalInput").ap()
    # qkv weights x32 packed [p, pp, s, m*128+j] = w[row(m,j), (pp*2+s)*128+p]
    wq8_io = nc.dram_tensor("wq8", [128, 4, 2, 384], E4, kind="ExternalInput").ap()
    # alibi*AL_SCALE transposed, Act k-tiles only: al8[hl, k, q], k < NAK*128
    al8_t = nc.dram_tensor("al8", [HPC, NAK * 128, T], E4, kind="ExternalInput")
    # bit-trick alibi bias, f16: ab16[hl, k - NAK*128, q]
    ab16_t = nc.dram_tensor("ab16", [HPC, NBK * 128, T], F16,
                            kind="ExternalInput")
    # identity of value IDENT_V for the alibi inject matmul
    i240_io = nc.dram_tensor("i240", [128, 128], E4, kind="ExternalInput").ap()
    identb_io = nc.dram_tensor("identb", [128, 128], BF, kind="ExternalInput").ap()
    # out-proj weights x32 packed [p, pp, s, dcol] = WS*w_out[dcol, (pp*2+s)*128+p]
    wo8_io = nc.dram_tensor("wo8", [128, 4, 2, D], E4, kind="ExternalInput").ap()
    x_res_io = nc.dram_tensor("x_res", [CHUNK, D], F32, kind="ExternalInput").ap()
    b_outT_io = nc.dram_tensor("b_outT", [1, D], F32, kind="ExternalInput").ap()
    # mlp-in weights, hi/lo packed in one tensor:
    # [p, ff, hl, kk, fin] = hi/lo of (32*w_in_eff)[ff*128+fin, kk*128+p]
    w8i_io = nc.dram_tensor("w8i", [128, 32, 2, 8, 128], E4,
                            kind="ExternalInput").ap()
    b_inT_io = nc.dram_tensor("b_inT", [128, 32], F32, kind="ExternalInput").ap()
    # mlp-out weights, hi/lo packed: [ffp, p, hl, s, dcol] =
    # hi/lo of (32*w_mlp_out)[dcol, (ffp*2+s)*128+p]
    w8o_t = nc.dram_tensor("w8o", [16, 128, 2, 2, D], E4,
                           kind="ExternalInput")
    out_io = nc.dram_tensor("out", [CHUNK, D], F32, kind="ExternalOutput").ap()

    # ---- internal DRAM ----
    # per-head-half collective buffers: A2A#0 fires after the hl=0 passes
    # and overlaps the hl=1 attention; A2A#1 covers the rest.
    cc_send = [nc.dram_tensor(f"cc_send{hl}", [D // 2, CHUNK], E4)
               for hl in range(2)]
    cc_recv = [nc.dram_tensor(f"cc_recv{hl}", [D // 2, CHUNK], E4)
               for hl in range(2)]

    with tile.TileContext(nc) as tc:
        with tc.tile_pool(name="consts", bufs=1) as consts:
            i240 = consts.tile([128, 128], E4, tag="i240")
            nc.sync.dma_start(i240[:], i240_io)
            identb = consts.tile([128, 128], BF, tag="identb")
            nc.sync.dma_start(identb[:], identb_io)
            warm_f = consts.tile([128, 512], F32, tag="warm_f")
            nc.vector.memset(warm_f[:], 0.5)
            warm_rhs = consts.tile([128, 512], F32R, tag="warm_rhs")
            nc.vector.tensor_copy(warm_rhs[:], warm_f[:])
            identr = consts.tile([128, 128], F32R, tag="identr")
            nc.vector.tensor_copy(identr[:], identb[:])
            # 32-scaled f32r identity: injects the (unscaled) y2 residual
            # into the x32-scaled mlp-out PSUM
            identr32 = consts.tile([128, 128], F32R, tag="identr32")
            nc.vector.tensor_scalar(identr32[:], identb[:], WS, None, MUL)
            # phase-3 input tiles (loaded mid-attention; consts pool lives
            # for the whole kernel)
            wo8 = consts.tile([128, 4, 2, D], E4, tag="wo8")
            b_in = consts.tile([128, 32], F32, tag="b_in")
            bob = consts.tile([1, D], F32, tag="bob")
            xrs = consts.tile([128, 4, D], F32, tag="xr")
            # f32r copy for the residual-inject matmul (fp32r operands must
            # be produced rounded; idle-Pool copies during attention)
            xrsr = consts.tile([128, 4, D], F32R, tag="xrr")
            # re-sharded attention output, assembled per head-half as each
            # AllToAll lands (partitions 0-63 <- hl=0, 64-127 <- hl=1)
            yrT = consts.tile([128, 8, 512], E4, tag="yrT")

            with tc.tile_pool(name="qkv", bufs=1) as qkv, \
                 tc.tile_pool(name="alb", bufs=3) as albp, \
                 tc.tile_pool(name="abb", bufs=3) as abbp:
                al_cache = {}

                def load_alibi(hl, qc, ring):
                    # ring choice controls WHEN the transfer runs relative
                    # to the x8 loads on the shared DMA engines: hl=0 goes
                    # in-order on the sync ring right after x8; hl=1 goes on
                    # the Act ring, emitted late in the projection so its
                    # seq-ordered DGE cannot race ahead of the x8 transfers
                    alt = albp.tile([128, NAK, 1024], E4, tag="al",
                                    name=f"al{hl}_{qc}")
                    ring.dma_start(
                        alt[:],
                        bass.AP(tensor=al8_t,
                                offset=hl * NAK * 128 * T + qc * 1024,
                                ap=[[T, 128], [128 * T, NAK], [1, 1024]]))
                    abt = abbp.tile([128, NBK, 1024], F16, tag="ab",
                                    name=f"ab{hl}_{qc}")
                    ring.dma_start(
                        abt[:],
                        bass.AP(tensor=ab16_t,
                                offset=hl * NBK * 128 * T + qc * 1024,
                                ap=[[T, 128], [128 * T, NBK], [1, 1024]]))
                    al_cache[(hl, qc)] = (alt, abt)

                q8s, k8s, v8s = [], [], []
                for b in range(2):
                    q8 = qkv.tile([128, 2, T], E4, tag=f"q8{b}", name=f"q8{b}")
                    k8 = qkv.tile([128, 2, T], E4, tag=f"k8{b}", name=f"k8{b}")
                    # v8 [p, ktpair, slot, hl, 128]: cols 0-63 v, col 64 = WS
                    # (softmax denominator), cols 65-127 zero (DR stationary
                    # width must be 32/64/128); zeroed first - the v8 copies
                    # need them before the q8/k8 zero-slots are read
                    v8 = qkv.tile([128, 8, 2, 2, 128], E4, tag=f"v8{b}",
                                  name=f"v8{b}")
                    nc.gpsimd.memset(v8[:, :, :, :, 64:], 0.0)
                    nc.vector.memset(v8[:, :, :, :, 64:65], WS)
                    q8s.append(q8); k8s.append(k8); v8s.append(v8)
                for b in range(2):
                    # zero-slot for scores DR second k-tile
                    nc.gpsimd.memset(k8s[b][:, 1, :], 0.0)
                    nc.gpsimd.memset(q8s[b][:, 1, :], 0.0)

                # ---------------- phase 1: QKV projection ----------------
                with tc.tile_pool(name="p1x", bufs=2) as p1x, \
                     tc.tile_pool(name="p1w", bufs=1) as p1w, \
                     tc.tile_pool(name="p1t", bufs=3) as p1t, \
                     tc.tile_pool(name="p1ps", bufs=4, space="PSUM") as p1ps, \
                     tc.tile_pool(name="p1pt", bufs=2, space="PSUM") as p1pt:
                    wq8 = p1w.tile([128, 4, 2, 384], E4, tag="wq8")
                    nc.sync.dma_start(wq8[:], wq8_io)
                    for wi_ in range(4):
                        wps = p1pt.tile([128, 512], F32, tag="pt",
                                        name=f"warms{wi_}")
                        nc.tensor.matmul(wps[:], identr[:], warm_rhs[:],
                                         start=True, stop=True)

                    # x loads first (proj critical path), alibi loads woven
                    # in between so they land before their passes without
                    # delaying the projection on the shared DMA engines
                    xtss = []
                    for b in range(2):
                        xts = p1x.tile([128, 4, 2, 2048], E4, tag="xt",
                                       name=f"xt{b}")
                        xtss.append(xts)
                        for cc4 in range(4):
                            nc.sync.dma_start(
                                xts[:, :, :, cc4 * 512:(cc4 + 1) * 512],
                                x8_io[:, :, :,
                                      b * 2048 + cc4 * 512:
                                      b * 2048 + (cc4 + 1) * 512])
                        if b == 0:
                            # first alibi pair ahead of x(b=1): pass 0 can
                            # start as soon as the b=0 projection lands
                            load_alibi(0, 0, nc.sync)
                    load_alibi(0, 1, nc.sync)

                    for b in range(2):
                        q8, k8, v8 = q8s[b], k8s[b], v8s[b]
                        xts = xtss[b]
                        with nc.named_scope(f"qkvproj{b}"):
                            for t in range(4):
                                tsl = slice(t * 512, (t + 1) * 512)
                                for m in range(3):   # q, k, v
                                    ps = p1ps.tile([128, 512], F32, tag="proj",
                                                   name=f"proj{b}_{t}_{m}")
                                    for pp in range(4):
                                        nc.tensor.matmul(
                                            ps[:],
                                            wq8[:, pp, :, m * 128:(m + 1) * 128],
                                            xts[:, pp, :, tsl],
                                            start=(pp == 0), stop=(pp == 3),
                                            perf_mode=DR)
                                    if m == 0:
                                        nc.scalar.copy(q8[:, 0, tsl], ps[:])
                                    elif m == 1:
                                        nc.vector.tensor_copy(k8[:, 0, tsl],
                                                              ps[:])
                                    else:
                                        vt = p1t.tile([128, 512], BF, tag="vt",
                                                      name=f"vt{b}_{t}")
                                        nc.scalar.copy(vt[:], ps[:])
                                        for j in range(4):
                                            ti = t * 4 + j
                                            pt = p1pt.tile([128, 128], BF,
                                                           tag="pt",
                                                           name=f"pt{b}_{ti}")
                                            nc.tensor.transpose(
                                                pt[:],
                                                vt[:, j * 128:(j + 1) * 128],
                                                identb[:])
                                            nc.vector.tensor_copy(
                                                v8[:, ti // 2, ti % 2, :, 0:64],
                                                pt[:].rearrange(
                                                    "p (h d) -> p h d", h=2))
                        if b == 1:
                            # sync ring: the pool rotation (bufs=3) holds
                            # the 4th tile until pass (0,0,1) completes, so
                            # these transfers naturally run mid-attention
                            # without competing with the x8/al(0,*) loads;
                            # nothing urgent sits behind them on SP
                            load_alibi(1, 0, nc.sync)
                            load_alibi(1, 1, nc.sync)

                # ---------------- phase 2: attention ----------------
                with nc.named_scope("attn"), \
                     tc.tile_pool(name="exps", bufs=6) as expp, \
                     tc.tile_pool(name="sps", bufs=3, space="PSUM") as spsp, \
                     tc.tile_pool(name="yups", bufs=1, space="PSUM") as yupp, \
                     tc.tile_pool(name="nrm", bufs=2) as nrmp, \
                     tc.tile_pool(name="yns", bufs=4) as ynp:
                    # each pass's final yn multiply is emitted mid-NEXT-pass
                    # so the in-order DVE queue never stalls on the Pool
                    # broadcast it depends on
                    pending_norm = []

                    def flush_norm():
                        while pending_norm:
                            pending_norm.pop(0)()

                    def attn_pass(hl, qc, b):
                        alt, abt = al_cache[(hl, qc)]
                        hsl = slice(hl * 64, (hl + 1) * 64)
                        yu = yupp.tile([128, 1024], F32, tag="yu",
                                       name=f"yu{hl}_{qc}_{b}")
                        ex8 = None
                        exq = []  # deferred attnV ops: (ktp, ex8)
                        for kt in range(KT):
                            is_act = kt in A_POS
                            sp = spsp.tile([128, 1024], F32, tag="sp",
                                           name=f"sp{hl}_{qc}_{b}_{kt}")
                            for qh in range(2):
                                qsl = slice(qc * 1024 + qh * 512,
                                            qc * 1024 + (qh + 1) * 512)
                                osl = slice(qh * 512, (qh + 1) * 512)
                                nc.tensor.matmul(
                                    sp[:, osl],
                                    k8s[b][hsl, :, kt * 128:(kt + 1) * 128],
                                    q8s[b][hsl, :, qsl],
                                    start=True, stop=not is_act,
                                    perf_mode=DR)
                                if is_act:
                                    nc.tensor.matmul(
                                        sp[:, osl],
                                        i240[:],
                                        alt[:, A_POS[kt], osl],
                                        start=False, stop=True)
                            if kt % 2 == 0:
                                ex8 = expp.tile([128, 2, 1024], E4, tag="ex",
                                                name=f"ex{hl}_{qc}_{b}_{kt}")
                            if is_act:
                                nc.scalar.activation(ex8[:, kt % 2, :], sp[:],
                                                     AF.Exp, scale=EXP_SCALE)
                            else:
                                nc.vector.scalar_tensor_tensor(
                                    ex8[:, kt % 2, :].bitcast(I8), sp[:],
                                    BT_SCALE, abt[:, D_POS[kt], :], MUL, ADD)
                            if kt == 5:
                                # previous pass's deferred yn multiply: by
                                # now its Pool broadcast has finished, so
                                # the DVE never blocks on it
                                flush_norm()
                            if kt % 2 == 1:
                                exq.append((kt // 2, ex8))
                            # drain attnV four k-tile-pairs behind the scores
                            # so the PE queue never waits on a fresh exp nor
                            # on the previous pass's norm freeing yu
                            while exq and (exq[0][0] <= kt // 2 - 4
                                           or kt == KT - 1):
                                ktp, exd = exq.pop(0)
                                for qh in range(2):
                                    osl = slice(qh * 512, (qh + 1) * 512)
                                    nc.tensor.matmul(
                                        yu[:, osl],
                                        v8s[b][:, ktp, :, hl, :],
                                        exd[:, :, osl],
                                        start=(ktp == 0), stop=(ktp == 7),
                                        perf_mode=DR)
                        rec = nrmp.tile([1, 1024], F32, tag="rec",
                                        name=f"rec{hl}_{qc}_{b}")
                        nc.vector.reciprocal(rec[:], yu[64:65, :])
                        bc = nrmp.tile([64, 1024], F32, tag="bc",
                                       name=f"bc{hl}_{qc}_{b}")
                        nc.gpsimd.partition_broadcast(bc[:], rec[:])

                        def norm_fin(hl=hl, qc=qc, b=b, yu=yu, bc=bc):
                            i = b * 2 + qc
                            yn = ynp.tile([64, 1024], E4, tag="yn",
                                          name=f"yn{hl}_{qc}_{b}")
                            nc.vector.tensor_mul(yn[:], yu[0:64, :], bc[:])
                            nc.sync.dma_start(
                                bass.AP(tensor=cc_send[hl],
                                        offset=2 * i * 64 * 512,
                                        ap=[[512, 64], [64 * 512, 2],
                                            [1, 512]]),
                                yn[:].rearrange("p (h c) -> p h c", h=2))
                        pending_norm.append(norm_fin)

                    def run_a2a(hl):
                        flush_norm()
                        with nc.named_scope(f"a2a{hl}"):
                            if sim1:
                                nc.sync.dma_start(cc_recv[hl][:],
                                                  cc_send[hl][:])
                            else:
                                nc.gpsimd.collective_compute(
                                    "AllToAll", mybir.AluOpType.bypass,
                                    replica_groups=[list(range(NCORES))],
                                    ins=[cc_send[hl][:]],
                                    outs=[cc_recv[hl][:]])
                            # assemble the hl half of yrT: partitions
                            # hl*64.. <- recv blocks [64, 512] per src core
                            nc.scalar.dma_start(
                                yrT[hl * 64:(hl + 1) * 64, :, :],
                                bass.AP(tensor=cc_recv[hl], offset=0,
                                        ap=[[512, 64], [64 * 512, 8],
                                            [1, 512]]))

                    def pe_keepwarm(n, pool, tag, name):
                        # bridge PE idle gaps so the p-state never drops
                        wt = pool.tile([128, 1024], F32, tag=tag, name=name)
                        for i_ in range(n):
                            nc.tensor.matmul(wt[:, 0:512], identr[:],
                                             warm_rhs[:],
                                             start=True, stop=True)

                    first = True
                    for hl in range(2):
                        for qc in range(2):
                            for b in range(2):
                                attn_pass(hl, qc, b)
                                if first:
                                    # no-dependency phase-3 loads: issued here
                                    # so they transfer during attention
                                    nc.sync.dma_start(wo8[:], wo8_io[:])
                                    nc.sync.dma_start(b_in[:], b_inT_io[:])
                                    nc.sync.dma_start(bob[:], b_outT_io[:])
                                    x_res_r = x_res_io.rearrange(
                                        "(t p) d -> p t d", p=128)
                                    for tt_ in range(4):
                                        nc.sync.dma_start(xrs[:, tt_, :],
                                                          x_res_r[:, tt_, :])
                                        nc.gpsimd.tensor_copy(
                                            xrsr[:, tt_, :], xrs[:, tt_, :])
                                    first = False
                        if hl == 1:
                            pe_keepwarm(20, spsp, "sp", "warm_a2a")
                        run_a2a(hl)

            # ---------------- phase 3: out-proj + LN + MLP ----------------
            with nc.named_scope("mlp"), \
                 tc.tile_pool(name="p3sb", bufs=1) as p3sb, \
                 tc.tile_pool(name="p3r", bufs=3) as p3r, \
                 tc.tile_pool(name="p3s", bufs=4) as p3s, \
                 tc.tile_pool(name="mlpw", bufs=6) as mlpw:
                # dummy Sqrt during the A2A hole: pulls the Act
                # function-table swap off the preamble critical path
                dum = p3s.tile([128, 1], F32, tag="dum")
                nc.scalar.activation(dum[:], warm_f[:, 0:1], AF.Sqrt,
                                     scale=1.0)
                bb = p3sb.tile([128, D], F32, tag="bb")
                nc.gpsimd.partition_broadcast(bb[:], bob[:])

                y_sb = p3sb.tile([128, 4, D], F32, tag="y_sb")
                # f32r: written rounded by the Pool stt, consumed by the
                # final-residual inject matmul
                y2_sb = p3sb.tile([128, 4, D], F32R, tag="y2_sb")
                hns = p3sb.tile([128, 4, D], BF, tag="hns")
                hT8 = p3sb.tile([128, 8, 512], E4, tag="hT8")
                eps = p3sb.tile([128, 1], F32, tag="eps")
                nc.vector.memset(eps[:], 1e-5)
                # 2-stage out-proj: the hl=0 partials (partitions 0-63 of
                # each pp slot) run as soon as A2A#0 lands, overlapping the
                # A2A#1 transfer; stage 2 accumulates the hl=1 rows and
                # injects the x32-scaled residual via an f32r identity
                # matmul, so the readout is a pure scaled copy.
                with tc.tile_pool(name="p3acc8", bufs=8,
                                  space="PSUM") as p3acc8:
                    pss_op = {}
                    for hstage in range(2):
                        psl = slice(hstage * 64, (hstage + 1) * 64)
                        for tt in range(4):
                            for dc in range(2):
                                if hstage == 0:
                                    pss_op[(tt, dc)] = p3acc8.tile(
                                        [128, 512], F32, tag="acc",
                                        name=f"acc{tt}_{dc}")
                                ps = pss_op[(tt, dc)]
                                for pp in range(4):
                                    nc.tensor.matmul(
                                        ps[:],
                                        yrT[psl, 2 * pp:2 * pp + 2,
                                            tt * 128:(tt + 1) * 128],
                                        wo8[psl, pp, :,
                                            dc * 512:(dc + 1) * 512],
                                        start=(hstage == 0 and pp == 0),
                                        stop=False,
                                        perf_mode=DR)
                                if hstage == 1:
                                    nc.tensor.matmul(
                                        ps[:], identr[:],
                                        xrsr[:, tt, dc * 512:(dc + 1) * 512],
                                        start=False, stop=True)
                    # readouts interleaved with the LN chains (dc0 on Act,
                    # dc1 on DVE) so the in-order engine queues dovetail;
                    # warm matmuls into just-read accumulators keep the PE
                    # p-state up through this window
                    for tt in range(4):
                        nc.scalar.activation(
                            y_sb[:, tt, 0:512], pss_op[(tt, 0)][:],
                            AF.Copy, scale=1.0 / WS)
                        nc.vector.tensor_scalar(
                            y_sb[:, tt, 512:1024], pss_op[(tt, 1)][:],
                            1.0 / WS, None, MUL)
                        for dc in range(2):
                            # y2 = y + b_mlp_out on the idle Pool engine
                            # (f32r out; the fin inject applies the x32 via
                            # the scaled identity)
                            nc.gpsimd.tensor_add(
                                y2_sb[:, tt, dc * 512:(dc + 1) * 512],
                                y_sb[:, tt, dc * 512:(dc + 1) * 512],
                                bb[:, dc * 512:(dc + 1) * 512])
                        for i_ in range(4):
                            nc.tensor.matmul(pss_op[(tt, 0)][:], identr[:],
                                             warm_rhs[:],
                                             start=True, stop=True)
                        stats = p3s.tile([128, 2, 6], F32, tag="stats")
                        for g in range(2):
                            nc.vector.bn_stats(
                                stats[:, g, :],
                                y_sb[:, tt, g * 512:(g + 1) * 512])
                        mv = p3s.tile([128, 2], F32, tag="mv")
                        nc.vector.bn_aggr(mv[:], stats[:])
                        sd = p3s.tile([128, 1], F32, tag="sd")
                        nc.scalar.activation(sd[:], mv[:, 1:2], AF.Sqrt,
                                             bias=eps[:], scale=1.0)
                        rstd = p3s.tile([128, 1], F32, tag="rstd")
                        nc.vector.reciprocal(rstd[:], sd[:])
                        nb2 = p3s.tile([128, 1], F32, tag="nb2")
                        nc.vector.scalar_tensor_tensor(
                            nb2[:], mv[:, 0:1], -1.0, rstd[:],
                            MUL, MUL)
                        nc.scalar.activation(hns[:, tt, :], y_sb[:, tt, :],
                                             AF.Identity,
                                             bias=nb2[:], scale=rstd[:])

                # transpose h_norm -> hT8 + hTlo (hi/lo fp8 split: h ~
                # hT8 + hTlo to bf16-class accuracy), then the MLP
                ln_mlp = tc.tile_pool(name="p3acc", bufs=2, space="PSUM")
                p3acc = ln_mlp.__enter__()
                p3pt_ctx = tc.tile_pool(name="p3pt", bufs=2, space="PSUM")
                p3pt = p3pt_ctx.__enter__()
                # prefetch the first mlp-in weight tiles during LN
                wis = {}
                for ff in range(5):
                    wis[ff] = mlpw.tile([128, 2, 8, 128], E4, tag="wi",
                                        name=f"wi{ff}")
                    nc.sync.dma_start(wis[ff][:], w8i_io[:, ff, :, :, :])
                # dc-major: the first mlp-in matmul contracts dc blocks 0-1
                # over ALL tokens, so this order lets the MLP start as soon
                # as the first two dc columns are transposed
                wtm = p3acc.tile([128, 512], F32, tag="acc", name="warmmlp")
                for dc in range(8):
                    for i_ in range(3):   # keep the PE p-state up
                        nc.tensor.matmul(wtm[:], identr[:], warm_rhs[:],
                                         start=True, stop=True)
                    for tt in range(4):
                        pt = p3pt.tile([128, 128], BF, tag="pt3")
                        nc.tensor.transpose(
                            pt[:], hns[:, tt, dc * 128:(dc + 1) * 128],
                            identb[:])
                        tsl = slice(tt * 128, (tt + 1) * 128)
                        # split the psum->fp8 copies across Act and DVE
                        if tt % 2 == 0:
                            nc.scalar.copy(hT8[:, dc, tsl], pt[:])
                        else:
                            nc.vector.tensor_copy(hT8[:, dc, tsl], pt[:])

                # transpose pool freed -> 6-buf mlp-out accumulators so
                # the dc=1 group starts while dc=0 drains
                p3pt_ctx.__exit__(None, None, None)
                p3mo_ctx = tc.tile_pool(name="p3mo", bufs=6, space="PSUM")
                p3mo = p3mo_ctx.__enter__()

                # MLP in + gelu, 3-pass hi/lo DR
                hmT8 = p3sb.tile([128, 32, 512], E4, tag="hmT8")
                hmTlo = p3sb.tile([128, 32, 512], E4, tag="hmTlo")
                for ff in range(32):
                    if ff in wis:
                        wi = wis[ff]
                    else:
                        wi = mlpw.tile([128, 2, 8, 128], E4, tag="wi",
                                       name=f"wi{ff}")
                        nc.sync.dma_start(wi[:], w8i_io[:, ff, :, :, :])
                    ps = p3acc.tile([128, 512], F32, tag="acc")
                    # 2-pass: h8 @ (Wh + Wl) - W at bf16-class accuracy, h
                    # at fp8 (errstudy2 V4: 1.2e-2 vs the 2e-2 gate)
                    first, last = (0, 0), (1, 3)
                    for pi, (wsl, h) in enumerate(
                            ((0, hT8), (1, hT8))):
                        for pp in range(4):
                            nc.tensor.matmul(
                                ps[:], wi[:, wsl, 2 * pp:2 * pp + 2, :],
                                h[:, 2 * pp:2 * pp + 2, :],
                                start=((pi, pp) == first),
                                stop=((pi, pp) == last), perf_mode=DR)
                        if ff == 0:
                            # fill PE idle while transposes/subs complete
                            for i_ in range(3):
                                nc.tensor.matmul(wtm[:], identr[:],
                                                 warm_rhs[:],
                                                 start=True, stop=True)
                    t2 = p3r.tile([128, 512], BF, tag="t2",
                                  name=f"t2_{ff}")
                    nc.scalar.activation(t2[:], ps[:], AF.Gelu,
                                         bias=b_in[:, ff:ff + 1],
                                         scale=1.0 / WS)
                    # hi/lo fp8 split of the gelu output for the DR mlp_out
                    nc.vector.tensor_copy(hmT8[:, ff, :], t2[:])
                    if ff % 2 == 0:
                        nc.vector.tensor_sub(hmTlo[:, ff, :], t2[:],
                                             hmT8[:, ff, :])
                    else:
                        nc.gpsimd.tensor_sub(hmTlo[:, ff, :], t2[:],
                                             hmT8[:, ff, :])

                # MLP out + final residual: 3-pass hi/lo DR over ff-pairs
                out_r = out_io.rearrange("(t p) d -> p t d", p=128)
                for dc in range(2):
                    pss = [p3mo.tile([128, 512], F32, tag="mo",
                                     name=f"mo{dc}_{i}") for i in range(4)]
                    for ffp in range(16):
                        w2 = mlpw.tile([128, 4, 512], E4, tag="w2",
                                       name=f"w2_{dc}_{ffp}")
                        nc.sync.dma_start(
                            w2[:],
                            bass.AP(tensor=w8o_t,
                                    offset=(ffp * 128 * 4 * D + dc * 512),
                                    ap=[[4 * D, 128], [D, 4], [1, 512]]))
                        for tt in range(4):
                            tsl = slice(tt * 128, (tt + 1) * 128)
                            for pi, (g, wsl) in enumerate(
                                    ((hmT8, 0), (hmT8, 1), (hmTlo, 0))):
                                nc.tensor.matmul(
                                    pss[tt][:],
                                    g[:, 2 * ffp:2 * ffp + 2, tsl],
                                    w2[:, 2 * wsl:2 * wsl + 2, :],
                                    start=(ffp == 0 and pi == 0),
                                    stop=False,
                                    perf_mode=DR)
                    for tt in range(4):
                        # inject y2 = WS*(y + b_out); psum carries x32 from
                        # the weight prescale, so one Act scaled copy
                        # finishes the kernel output
                        nc.tensor.matmul(
                            pss[tt][:], identr32[:],
                            y2_sb[:, tt, dc * 512:(dc + 1) * 512],
                            start=False, stop=True)
                        fin = p3s.tile([128, 512], F32, tag="fin")
                        nc.scalar.activation(fin[:], pss[tt][:], AF.Copy,
                                             scale=1.0 / WS)
                        nc.sync.dma_start(
                            out_r[:, tt, dc * 512:(dc + 1) * 512], fin[:])
                for ctx in (p3mo_ctx, ln_mlp):
                    ctx.__exit__(None, None, None)

    nc.compile()
    return nc


def _host_prep(x, alibi, ln1_w, w_qkv, w_out, ln2_w, w_mlp_in, b_mlp_in,
               w_mlp_out, b_mlp_out):
    f32 = np.float32
    x = np.asarray(x, f32)
    x_flat = np.ascontiguousarray(x.reshape(NTOK, D))
    w_qkv = np.asarray(w_qkv, f32)
    w_out = np.asarray(w_out, f32)
    w_mlp_in = np.asarray(w_mlp_in, f32)
    w_mlp_out = np.asarray(w_mlp_out, f32)
    b_mlp_in = np.asarray(b_mlp_in, f32)
    b_mlp_out = np.asarray(b_mlp_out, f32)
    ln2_w = np.asarray(ln2_w, f32)
    alibi = np.asarray(alibi, f32)

    # x8 [p, pp, s, tok] = x[tok, (pp*2+s)*128 + p]
    x8 = np.ascontiguousarray(
        x_flat.T.reshape(4, 2, 128, NTOK).transpose(2, 0, 1, 3)).astype(E4M3)

    # identities
    i240 = (IDENT_V * np.eye(128, dtype=f32)).astype(E4M3)
    identb = np.eye(128, dtype=f32).astype(BF16)

    # out-proj weights: wo8[p, pp, s, dcol] = WS * w_out[dcol, (pp*2+s)*128+p]
    wo8 = np.ascontiguousarray(
        (WS * w_out).T.reshape(4, 2, 128, D).transpose(2, 0, 1, 3)
    ).astype(E4M3)

    w_in_eff = WS * w_mlp_in * ln2_w[None, :]     # (FF, D), x32 for fp8
    w_inP = np.ascontiguousarray(
        w_in_eff.reshape(32, 128, 8, 128).transpose(3, 0, 2, 1))
    w8i_h = w_inP.astype(E4M3)
    w8i_l = (w_inP - w8i_h.astype(np.float32)).astype(E4M3)
    # packed hi/lo: [p, ff, hl, kk, fin]
    w8i = np.ascontiguousarray(
        np.stack([w8i_h, w8i_l], axis=2))
    # w2P [ffp, p, s, dcol] = (32*w_mlp_out)[dcol, (ffp*2+s)*128+p]
    w2P = np.ascontiguousarray(
        (WS * w_mlp_out).T.reshape(16, 2, 128, D).transpose(0, 2, 1, 3))
    w8o_h = w2P.astype(E4M3)
    w8o_l = (w2P - w8o_h.astype(np.float32)).astype(E4M3)
    # packed hi/lo: [ffp, p, hl, s, dcol]
    w8o = np.ascontiguousarray(np.stack([w8o_h, w8o_l], axis=2))
    b_inT = np.ascontiguousarray(b_mlp_in.reshape(32, 128).T)

    in_maps = []
    for c in range(NCORES):
        h0 = HPC * c
        qrows = WS * w_qkv[h0 * Dh:(h0 + HPC) * Dh]
        krows = WS * w_qkv[H * Dh + h0 * Dh:H * Dh + (h0 + HPC) * Dh]
        vrows = WS * w_qkv[2 * H * Dh + h0 * Dh:2 * H * Dh + (h0 + HPC) * Dh]
        wq = np.concatenate([qrows, krows, vrows], 0)   # [384, D]
        # wq8 [p, pp, s, mcol] = wq[mcol, (pp*2+s)*128 + p]
        wq8 = np.ascontiguousarray(
            wq.T.reshape(4, 2, 128, 384).transpose(2, 0, 1, 3)).astype(E4M3)
        # alibi transposed to [hl, k, q]; k-row blocks gathered per engine
        # assignment (al8 row-block i = ACT_KTS[i], ab16 row-block i =
        # DVE_KTS[i])
        alc = alibi[0, h0:h0 + HPC].transpose(0, 2, 1)  # [HPC, T(k), T(q)]
        alc_kt = alc.reshape(HPC, KT, 128, T)
        al8 = np.ascontiguousarray(
            AL_SCALE * alc_kt[:, ACT_KTS].reshape(HPC, NAK * 128, T)
        ).astype(E4M3)
        ab16 = np.ascontiguousarray(
            LOG2E8 * alc_kt[:, DVE_KTS].reshape(HPC, NBK * 128, T) + BT_BIAS
        ).astype(np.float16)
        # residual pre-scaled x32: it is injected into the x32-scaled
        # out-proj / mlp-out PSUMs via f32r identity matmuls
        x_res = np.ascontiguousarray(WS * x_flat[c * CHUNK:(c + 1) * CHUNK])
        in_maps.append({
            "x8": x8, "wq8": wq8, "al8": al8, "ab16": ab16, "i240": i240,
            "identb": identb, "wo8": wo8,
            "x_res": x_res, "b_outT": b_mlp_out.reshape(1, D),
            "w8i": w8i, "b_inT": b_inT, "w8o": w8o,
        })
    return in_maps


def _get_compiled():
    global _COMPILED
    if _COMPILED is None:
        _COMPILED = _build()
    return _COMPILED


def kernel(_trace=False, **inputs):
    nc = _get_compiled()
    in_maps = _host_prep(**inputs)
    res = None
    for attempt in range(3):
        try:
            res = run_bass_kernel_spmd(nc, in_maps,
                                       core_ids=list(range(NCORES)),
                                       trace=_trace)
            break
        except Exception:
            if attempt == 2:
                raise
    out = np.concatenate([res.results[c]["out"] for c in range(NCORES)], 0)
    out = out.reshape(B, T, D).astype(np.float32)
    if _trace:
        return out, res
    return out


# revision 65
# speedup vs baseline: 1.2484x; 1.0423x over previous
"""Trainium2 Bass kernel for nn_AttentionBlock (B=2, T=2048, D=1024, H=16,
Dh=64, Ff=4096), SPMD across 8 NeuronCores in one NEFF launch.

Sharding:
  - Phase 1+2 (QKV projection + attention): 2 heads per core over all 4096
    tokens. Phase 3 (out-proj + residual + LayerNorm + MLP): 512 tokens per
    core, re-sharded via an AllToAll of the attention output (fp8, 0.5 MiB).

Numerics (validated end-to-end in numpy against the reference, see
errstudy2.py: ~2e-3 max-rel vs the 2e-2 gate):
  - Attention matmuls run fp8e4m3 DoubleRow as before (QKV projection,
    Q.K^T scores, attn@V, output projection). Weights pre-scaled x32.
  - The softmax exp is split across two engines per k-tile (the Act engine
    was the attention bottleneck at ~1us per [128,1024] exp):
      * kt < NAK: alibi injected into the scores PSUM by a plain fp8 matmul
        against a 240-valued identity (as before), then Act exp ->
        fp8 attention weights.
      * kt >= NAK: DVE computes the fp8 weights directly via the
        exp bit-trick: i8 = psum*(8*log2e/8192) + (alibi*8*log2e + 56 [+rnd])
        written as int8 and bit-viewed as e4m3 ~= exp(s+alibi). The f16
        alibi-bias tile fuses the alibi add, so no PE inject is needed.
    The bit-trick's mantissa-interpolation error (~3%) is the same class as
    the e4m3 quantization the Act path already incurs; softmax averaging
    washes both out (errstudy2: 2.01e-3 vs 2.13e-3).
  - Scores DoubleRow uses a zeroed second k-tile slot; attn@V pairs adjacent
    score k-tiles. V padded to 128 cols (col 64 = softmax-denominator ones
    column = 32).
  - mlp_in runs 2-pass (h8 @ (W_hi + W_lo): weights at bf16-class accuracy,
    activations fp8); mlp_out stays 3-pass hi/lo. End-to-end rel_absmax
    measured 1.204e-2 on HW (errstudy2 V4 predicted 1.206e-2) vs the 2e-2
    gate. The A2A is split by head-half so the first collective overlaps
    the second half of attention, and the out-projection runs in two
    partition-half stages with the residual injected via f32r identity
    matmuls (readouts become pure Act/DVE scaled copies).

kernel(**inputs) takes FULL unsharded inputs, returns the FULL output.
"""

import sys

for _p in ("/opt/trn_rl_repo", "/root/.axon_site/_ro/trn_rl_repo"):
    if _p not in sys.path:
        sys.path.insert(0, _p)

import numpy as np
import ml_dtypes

import concourse.bass as bass
import concourse.tile as tile
from concourse import bacc, mybir
from concourse.bass_utils import run_bass_kernel_spmd

BF16 = ml_dtypes.bfloat16
E4M3 = ml_dtypes.float8_e4m3   # TRN float8e4: max 240

B, T, D, H, Dh, FF = 2, 2048, 1024, 16, 64, 4096
NTOK = B * T            # 4096
NCORES = 8
CHUNK = NTOK // NCORES  # 512 tokens per core
HPC = H // NCORES       # 2 heads per core
KT = T // 128           # 16 k-tiles per batch

# Interleaved engine assignment of the 16 k-tiles per pass: Act tiles get
# true exp (+PE alibi inject), DVE tiles get the fused bit-trick exp.
# Interleaving keeps both engines fed as the PE emits scores in kt order.
ACT_KTS = (0, 1, 3, 4, 6, 8, 10, 12, 14, 15)
DVE_KTS = (2, 5, 7, 9, 11, 13)
NAK = len(ACT_KTS)
NBK = len(DVE_KTS)
A_POS = {kt: i for i, kt in enumerate(ACT_KTS)}
D_POS = {kt: i for i, kt in enumerate(DVE_KTS)}

WS = 32.0               # host weight pre-scale for fp8 range
EXP_SCALE = 1.0 / (WS * WS * 8.0)   # 1/8192: undo q,k x32 and apply 1/sqrt(Dh)
IDENT_V = 240.0                      # e4m3-exact identity magnitude
AL_SCALE = (WS * WS * 8.0) / IDENT_V  # alibi pre-scale so PSUM alibi = 8192*a
LOG2E8 = float(8.0 * np.log2(np.e))  # e4m3 bit-trick exp slope
BT_SCALE = LOG2E8 / 8192.0
BT_BIAS = 56.0          # e4m3 exponent-bias term (see _host_prep; rounding
                        # offset calibrated on HW via probe.py)

F32 = mybir.dt.float32
F32R = mybir.dt.float32r
BF = mybir.dt.bfloat16
E4 = mybir.dt.float8e4
F16 = mybir.dt.float16
I8 = mybir.dt.int8
AF = mybir.ActivationFunctionType
DR = mybir.MatmulPerfMode.DoubleRow
MUL = mybir.AluOpType.mult
ADD = mybir.AluOpType.add

_COMPILED = None


def _build(sim1=False):
    nc = bacc.Bacc("TRN2", target_bir_lowering=False, debug=False,
                   num_devices=1 if sim1 else NCORES)

    # ---- kernel I/O (per core) ----
    # x packed [p, pp, s, tok] = x[tok, (pp*2+s)*128 + p], fp8
    x8_io = nc.dram_tensor("x8", [128, 4, 2, NTOK], E4, kind="ExternalInput").ap()
    # qkv weights x32 packed [p, pp, s, m*128+j] = w[row(m,j), (pp*2+s)*128+p]
    wq8_io = nc.dram_tensor("wq8", [128, 4, 2, 384], E4, kind="ExternalInput").ap()
    # alibi*AL_SCALE transposed, Act k-tiles only: al8[hl, k, q], k < NAK*128
    al8_t = nc.dram_tensor("al8", [HPC, NAK * 128, T], E4, kind="ExternalInput")
    # bit-trick alibi bias, f16: ab16[hl, k - NAK*128, q]
    ab16_t = nc.dram_tensor("ab16", [HPC, NBK * 128, T], F16,
                            kind="ExternalInput")
    # identity of value IDENT_V for the alibi inject matmul
    i240_io = nc.dram_tensor("i240", [128, 128], E4, kind="ExternalInput").ap()
    identb_io = nc.dram_tensor("identb", [128, 128], BF, kind="ExternalInput").ap()
    # out-proj weights x32 packed [p, pp, s, dcol] = WS*w_out[dcol, (pp*2+s)*128+p]
    wo8_io = nc.dram_tensor("wo8", [128, 4, 2, D], E4, kind="ExternalInput").ap()
    x_res_io = nc.dram_tensor("x_res", [CHUNK, D], F32, kind="ExternalInput").ap()
    b_outT_io = nc.dram_tensor("b_outT", [1, D], F32, kind="ExternalInput").ap()
    # mlp-in weights, hi/lo packed in one tensor:
    # [p, ff, hl, kk, fin] = hi/lo of (32*w_in_eff)[ff*128+fin, kk*128+p]
    w8i_io = nc.dram_tensor("w8i", [128, 32, 2, 8, 128], E4,
                            kind="ExternalInput").ap()
    b_inT_io = nc.dram_tensor("b_inT", [128, 32], F32, kind="ExternalInput").ap()
    # mlp-out weights, hi/lo packed: [ffp, p, hl, s, dcol] =
    # hi/lo of (32*w_mlp_out)[dcol, (ffp*2+s)*128+p]
    w8o_t = nc.dram_tensor("w8o", [16, 128, 2, 2, D], E4,
                           kind="ExternalInput")
    out_io = nc.dram_tensor("out", [CHUNK, D], F32, kind="ExternalOutput").ap()

    # ---- internal DRAM ----
    # per-head-half collective buffers: A2A#0 fires after the hl=0 passes
    # and overlaps the hl=1 attention; A2A#1 covers the rest.
    cc_send = [nc.dram_tensor(f"cc_send{hl}", [D // 2, CHUNK], E4)
               for hl in range(2)]
    cc_recv = [nc.dram_tensor(f"cc_recv{hl}", [D // 2, CHUNK], E4)
               for hl in range(2)]

    with tile.TileContext(nc) as tc:
        with tc.tile_pool(name="consts", bufs=1) as consts:
            i240 = consts.tile([128, 128], E4, tag="i240")
            nc.sync.dma_start(i240[:], i240_io)
            identb = consts.tile([128, 128], BF, tag="identb")
            nc.sync.dma_start(identb[:], identb_io)
            warm_f = consts.tile([128, 512], F32, tag="warm_f")
            nc.vector.memset(warm_f[:], 0.5)
            warm_rhs = consts.tile([128, 512], F32R, tag="warm_rhs")
            nc.vector.tensor_copy(warm_rhs[:], warm_f[:])
            identr = consts.tile([128, 128], F32R, tag="identr")
            nc.vector.tensor_copy(identr[:], identb[:])
            # 32-scaled f32r identity: injects the (unscaled) y2 residual
            # into the x32-scaled mlp-out PSUM
            identr32 = consts.tile([128, 128], F32R, tag="identr32")
            nc.vector.tensor_scalar(identr32[:], identb[:], WS, None, MUL)
            # phase-3 input tiles (loaded mid-attention; consts pool lives
            # for the whole kernel)
            wo8 = consts.tile([128, 4, 2, D], E4, tag="wo8")
            b_in = consts.tile([128, 32], F32, tag="b_in")
            bob = consts.tile([1, D], F32, tag="bob")
            xrs = consts.tile([128, 4, D], F32, tag="xr")
            # f32r copy for the residual-inject matmul (fp32r operands must
            # be produced rounded; idle-Pool copies during attention)
            xrsr = consts.tile([128, 4, D], F32R, tag="xrr")
            # re-sharded attention output, assembled per head-half as each
            # AllToAll lands (partitions 0-63 <- hl=0, 64-127 <- hl=1)
            yrT = consts.tile([128, 8, 512], E4, tag="yrT")

            with tc.tile_pool(name="qkv", bufs=1) as qkv, \
                 tc.tile_pool(name="alb", bufs=3) as albp, \
                 tc.tile_pool(name="abb", bufs=3) as abbp:
                al_cache = {}

                def load_alibi(hl, qc, ring):
                    # ring choice controls WHEN the transfer runs relative
                    # to the x8 loads on the shared DMA engines: hl=0 goes
                    # in-order on the sync ring right after x8; hl=1 goes on
                    # the Act ring, emitted late in the projection so its
                    # seq-ordered DGE cannot race ahead of the x8 transfers
                    alt = albp.tile([128, NAK, 1024], E4, tag="al",
                                    name=f"al{hl}_{qc}")
                    ring.dma_start(
                        alt[:],
                        bass.AP(tensor=al8_t,
                                offset=hl * NAK * 128 * T + qc * 1024,
                                ap=[[T, 128], [128 * T, NAK], [1, 1024]]))
                    abt = abbp.tile([128, NBK, 1024], F16, tag="ab",
                                    name=f"ab{hl}_{qc}")
                    ring.dma_start(
                        abt[:],
                        bass.AP(tensor=ab16_t,
                                offset=hl * NBK * 128 * T + qc * 1024,
                                ap=[[T, 128], [128 * T, NBK], [1, 1024]]))
                    al_cache[(hl, qc)] = (alt, abt)

                q8s, k8s, v8s = [], [], []
                for b in range(2):
                    q8 = qkv.tile([128, 2, T], E4, tag=f"q8{b}", name=f"q8{b}")
                    k8 = qkv.tile([128, 2, T], E4, tag=f"k8{b}", name=f"k8{b}")
                    # v8 [p, ktpair, slot, hl, 128]: cols 0-63 v, col 64 = WS
                    # (softmax denominator), cols 65-127 zero (DR stationary
                    # width must be 32/64/128); zeroed first - the v8 copies
                    # need them before the q8/k8 zero-slots are read
                    v8 = qkv.tile([128, 8, 2, 2, 128], E4, tag=f"v8{b}",
                                  name=f"v8{b}")
                    nc.gpsimd.memset(v8[:, :, :, :, 64:], 0.0)
                    nc.vector.memset(v8[:, :, :, :, 64:65], WS)
                    q8s.append(q8); k8s.append(k8); v8s.append(v8)
                for b in range(2):
                    # zero-slot for scores DR second k-tile
                    nc.gpsimd.memset(k8s[b][:, 1, :], 0.0)
                    nc.gpsimd.memset(q8s[b][:, 1, :], 0.0)

                # ---------------- phase 1: QKV projection ----------------
                with tc.tile_pool(name="p1x", bufs=2) as p1x, \
                     tc.tile_pool(name="p1w", bufs=1) as p1w, \
                     tc.tile_pool(name="p1t", bufs=3) as p1t, \
                     tc.tile_pool(name="p1ps", bufs=6, space="PSUM") as p1ps, \
                     tc.tile_pool(name="p1pt", bufs=2, space="PSUM") as p1pt:
                    wq8 = p1w.tile([128, 4, 2, 384], E4, tag="wq8")
                    nc.sync.dma_start(wq8[:], wq8_io)
                    for wi_ in range(4):
                        wps = p1pt.tile([128, 512], F32, tag="pt",
                                        name=f"warms{wi_}")
                        nc.tensor.matmul(wps[:], identr[:], warm_rhs[:],
                                         start=True, stop=True)

                    # x loads first (proj critical path), alibi loads woven
                    # in between so they land before their passes without
                    # delaying the projection on the shared DMA engines
                    xtss = []
                    for b in range(2):
                        xts = p1x.tile([128, 4, 2, 2048], E4, tag="xt",
                                       name=f"xt{b}")
                        xtss.append(xts)
                        for cc4 in range(4):
                            nc.sync.dma_start(
                                xts[:, :, :, cc4 * 512:(cc4 + 1) * 512],
                                x8_io[:, :, :,
                                      b * 2048 + cc4 * 512:
                                      b * 2048 + (cc4 + 1) * 512])
                    load_alibi(0, 0, nc.sync)
                    load_alibi(0, 1, nc.sync)

                    for b in range(2):
                        q8, k8, v8 = q8s[b], k8s[b], v8s[b]
                        xts = xtss[b]
                        with nc.named_scope(f"qkvproj{b}"):
                            for t in range(4):
                                tsl = slice(t * 512, (t + 1) * 512)
                                for m in range(3):   # q, k, v
                                    ps = p1ps.tile([128, 512], F32, tag="proj",
                                                   name=f"proj{b}_{t}_{m}")
                                    for pp in range(4):
                                        nc.tensor.matmul(
                                            ps[:],
                                            wq8[:, pp, :, m * 128:(m + 1) * 128],
                                            xts[:, pp, :, tsl],
                                            start=(pp == 0), stop=(pp == 3),
                                            perf_mode=DR)
                                    if m == 0:
                                        nc.scalar.copy(q8[:, 0, tsl], ps[:])
                                    elif m == 1:
                                        nc.vector.tensor_copy(k8[:, 0, tsl],
                                                              ps[:])
                                    else:
                                        vt = p1t.tile([128, 512], BF, tag="vt",
                                                      name=f"vt{b}_{t}")
                                        nc.scalar.copy(vt[:], ps[:])
                                        for j in range(4):
                                            ti = t * 4 + j
                                            pt = p1pt.tile([128, 128], BF,
                                                           tag="pt",
                                                           name=f"pt{b}_{ti}")
                                            nc.tensor.transpose(
                                                pt[:],
                                                vt[:, j * 128:(j + 1) * 128],
                                                identb[:])
                                            nc.vector.tensor_copy(
                                                v8[:, ti // 2, ti % 2, :, 0:64],
                                                pt[:].rearrange(
                                                    "p (h d) -> p h d", h=2))
                        if b == 1:
                            # sync ring: the pool rotation (bufs=3) holds
                            # the 4th tile until pass (0,0,1) completes, so
                            # these transfers naturally run mid-attention
                            # without competing with the x8/al(0,*) loads;
                            # nothing urgent sits behind them on SP
                            load_alibi(1, 0, nc.sync)
                            load_alibi(1, 1, nc.sync)

                # ---------------- phase 2: attention ----------------
                with nc.named_scope("attn"), \
                     tc.tile_pool(name="exps", bufs=8) as expp, \
                     tc.tile_pool(name="sps", bufs=3, space="PSUM") as spsp, \
                     tc.tile_pool(name="yups", bufs=1, space="PSUM") as yupp, \
                     tc.tile_pool(name="nrm", bufs=2) as nrmp, \
                     tc.tile_pool(name="yns", bufs=4) as ynp:
                    # each pass's final yn multiply is emitted mid-NEXT-pass
                    # so the in-order DVE queue never stalls on the Pool
                    # broadcast it depends on
                    pending_norm = []

                    def flush_norm():
                        while pending_norm:
                            pending_norm.pop(0)()

                    def attn_pass(hl, qc, b):
                        alt, abt = al_cache[(hl, qc)]
                        hsl = slice(hl * 64, (hl + 1) * 64)
                        yu = yupp.tile([128, 1024], F32, tag="yu",
                                       name=f"yu{hl}_{qc}_{b}")
                        ex8 = None
                        exq = []  # deferred attnV ops: (ktp, ex8)
                        for kt in range(KT):
                            is_act = kt in A_POS
                            sp = spsp.tile([128, 1024], F32, tag="sp",
                                           name=f"sp{hl}_{qc}_{b}_{kt}")
                            for qh in range(2):
                                qsl = slice(qc * 1024 + qh * 512,
                                            qc * 1024 + (qh + 1) * 512)
                                osl = slice(qh * 512, (qh + 1) * 512)
                                nc.tensor.matmul(
                                    sp[:, osl],
                                    k8s[b][hsl, :, kt * 128:(kt + 1) * 128],
                                    q8s[b][hsl, :, qsl],
                                    start=True, stop=not is_act,
                                    perf_mode=DR)
                                if is_act:
                                    nc.tensor.matmul(
                                        sp[:, osl],
                                        i240[:],
                                        alt[:, A_POS[kt], osl],
                                        start=False, stop=True)
                            if kt % 2 == 0:
                                ex8 = expp.tile([128, 2, 1024], E4, tag="ex",
                                                name=f"ex{hl}_{qc}_{b}_{kt}")
                            if is_act:
                                nc.scalar.activation(ex8[:, kt % 2, :], sp[:],
                                                     AF.Exp, scale=EXP_SCALE)
                            else:
                                nc.vector.scalar_tensor_tensor(
                                    ex8[:, kt % 2, :].bitcast(I8), sp[:],
                                    BT_SCALE, abt[:, D_POS[kt], :], MUL, ADD)
                            if kt == 5:
                                # previous pass's deferred yn multiply: by
                                # now its Pool broadcast has finished, so
                                # the DVE never blocks on it
                                flush_norm()
                            if kt % 2 == 1:
                                exq.append((kt // 2, ex8))
                            # drain attnV four k-tile-pairs behind the scores
                            # so the PE queue never waits on a fresh exp nor
                            # on the previous pass's norm freeing yu
                            while exq and (exq[0][0] <= kt // 2 - 4
                                           or kt == KT - 1):
                                ktp, exd = exq.pop(0)
                                for qh in range(2):
                                    osl = slice(qh * 512, (qh + 1) * 512)
                                    nc.tensor.matmul(
                                        yu[:, osl],
                                        v8s[b][:, ktp, :, hl, :],
                                        exd[:, :, osl],
                                        start=(ktp == 0), stop=(ktp == 7),
                                        perf_mode=DR)
                        rec = nrmp.tile([1, 1024], F32, tag="rec",
                                        name=f"rec{hl}_{qc}_{b}")
                        nc.vector.reciprocal(rec[:], yu[64:65, :])
                        bc = nrmp.tile([64, 1024], F32, tag="bc",
                                       name=f"bc{hl}_{qc}_{b}")
                        nc.gpsimd.partition_broadcast(bc[:], rec[:])

                        def norm_fin(hl=hl, qc=qc, b=b, yu=yu, bc=bc):
                            i = b * 2 + qc
                            yn = ynp.tile([64, 1024], E4, tag="yn",
                                          name=f"yn{hl}_{qc}_{b}")
                            nc.vector.tensor_mul(yn[:], yu[0:64, :], bc[:])
                            nc.sync.dma_start(
                                bass.AP(tensor=cc_send[hl],
                                        offset=2 * i * 64 * 512,
                                        ap=[[512, 64], [64 * 512, 2],
                                            [1, 512]]),
                                yn[:].rearrange("p (h c) -> p h c", h=2))
                        pending_norm.append(norm_fin)

                    def run_a2a(hl):
                        with nc.named_scope(f"a2a{hl}"):
                            if sim1:
                                nc.sync.dma_start(cc_recv[hl][:],
                                                  cc_send[hl][:])
                            else:
                                nc.gpsimd.collective_compute(
                                    "AllToAll", mybir.AluOpType.bypass,
                                    replica_groups=[list(range(NCORES))],
                                    ins=[cc_send[hl][:]],
                                    outs=[cc_recv[hl][:]])
                            # assemble the hl half of yrT: partitions
                            # hl*64.. <- recv blocks [64, 512] per src core
                            # (sync ring: a scalar-ring DGE would block the
                            # Act queue's exps until the collective lands)
                            nc.sync.dma_start(
                                yrT[hl * 64:(hl + 1) * 64, :, :],
                                bass.AP(tensor=cc_recv[hl], offset=0,
                                        ap=[[512, 64], [64 * 512, 8],
                                            [1, 512]]))

                    def pe_keepwarm(n, pool, tag, name):
                        # bridge PE idle gaps so the p-state never drops
                        wt = pool.tile([128, 1024], F32, tag=tag, name=name)
                        for i_ in range(n):
                            nc.tensor.matmul(wt[:, 0:512], identr[:],
                                             warm_rhs[:],
                                             start=True, stop=True)

                    first = True
                    for hl in range(2):
                        for qc in range(2):
                            for b in range(2):
                                attn_pass(hl, qc, b)
                                if first:
                                    # no-dependency phase-3 loads: issued here
                                    # so they transfer during attention
                                    nc.sync.dma_start(wo8[:], wo8_io[:])
                                    nc.sync.dma_start(b_in[:], b_inT_io[:])
                                    nc.sync.dma_start(bob[:], b_outT_io[:])
                                    x_res_r = x_res_io.rearrange(
                                        "(t p) d -> p t d", p=128)
                                    for tt_ in range(4):
                                        nc.sync.dma_start(xrs[:, tt_, :],
                                                          x_res_r[:, tt_, :])
                                        nc.gpsimd.tensor_copy(
                                            xrsr[:, tt_, :], xrs[:, tt_, :])
                                    first = False
                        if hl == 0:
                            # A2A#0 emission rides the deferred-norm queue:
                            # it lands right after pass 3's yn DMA, inside
                            # pass 4's DVE stream (no hl-boundary stall)
                            pending_norm.append(lambda: run_a2a(0))
                        else:
                            pe_keepwarm(20, spsp, "sp", "warm_a2a")
                            flush_norm()
                            run_a2a(1)

            # ---------------- phase 3: out-proj + LN + MLP ----------------
            with nc.named_scope("mlp"), \
                 tc.tile_pool(name="p3sb", bufs=1) as p3sb, \
                 tc.tile_pool(name="p3r", bufs=3) as p3r, \
                 tc.tile_pool(name="p3s", bufs=4) as p3s, \
                 tc.tile_pool(name="mlpw", bufs=8) as mlpw:
                # dummy Sqrt during the A2A hole: pulls the Act
                # function-table swap off the preamble critical path
                dum = p3s.tile([128, 1], F32, tag="dum")
                nc.scalar.activation(dum[:], warm_f[:, 0:1], AF.Sqrt,
                                     scale=1.0)
                bb = p3sb.tile([128, D], F32, tag="bb")
                nc.gpsimd.partition_broadcast(bb[:], bob[:])

                y_sb = p3sb.tile([128, 4, D], F32, tag="y_sb")
                # f32r: written rounded by the Pool stt, consumed by the
                # final-residual inject matmul
                y2_sb = p3sb.tile([128, 4, D], F32R, tag="y2_sb")
                hns = p3sb.tile([128, 4, D], BF, tag="hns")
                hT8 = p3sb.tile([128, 8, 512], E4, tag="hT8")
                eps = p3sb.tile([128, 1], F32, tag="eps")
                nc.vector.memset(eps[:], 1e-5)
                # 2-stage out-proj: the hl=0 partials (partitions 0-63 of
                # each pp slot) run as soon as A2A#0 lands, overlapping the
                # A2A#1 transfer; stage 2 accumulates the hl=1 rows and
                # injects the x32-scaled residual via an f32r identity
                # matmul, so the readout is a pure scaled copy.
                with tc.tile_pool(name="p3acc8", bufs=8,
                                  space="PSUM") as p3acc8:
                    pss_op = {}
                    for hstage in range(2):
                        psl = slice(hstage * 64, (hstage + 1) * 64)
                        for tt in range(4):
                            for dc in range(2):
                                if hstage == 0:
                                    pss_op[(tt, dc)] = p3acc8.tile(
                                        [128, 512], F32, tag="acc",
                                        name=f"acc{tt}_{dc}")
                                ps = pss_op[(tt, dc)]
                                for pp in range(4):
                                    nc.tensor.matmul(
                                        ps[:],
                                        yrT[psl, 2 * pp:2 * pp + 2,
                                            tt * 128:(tt + 1) * 128],
                                        wo8[psl, pp, :,
                                            dc * 512:(dc + 1) * 512],
                                        start=(hstage == 0 and pp == 0),
                                        stop=False,
                                        perf_mode=DR)
                                if hstage == 1:
                                    nc.tensor.matmul(
                                        ps[:], identr[:],
                                        xrsr[:, tt, dc * 512:(dc + 1) * 512],
                                        start=False, stop=True)
                    # readouts interleaved with the LN chains (dc0 on Act,
                    # dc1 on DVE) so the in-order engine queues dovetail;
                    # warm matmuls into just-read accumulators keep the PE
                    # p-state up through this window
                    for tt in range(4):
                        nc.scalar.activation(
                            y_sb[:, tt, 0:512], pss_op[(tt, 0)][:],
                            AF.Copy, scale=1.0 / WS)
                        nc.vector.tensor_scalar(
                            y_sb[:, tt, 512:1024], pss_op[(tt, 1)][:],
                            1.0 / WS, None, MUL)
                        for dc in range(2):
                            # y2 = y + b_mlp_out on the idle Pool engine
                            # (f32r out; the fin inject applies the x32 via
                            # the scaled identity)
                            nc.gpsimd.tensor_add(
                                y2_sb[:, tt, dc * 512:(dc + 1) * 512],
                                y_sb[:, tt, dc * 512:(dc + 1) * 512],
                                bb[:, dc * 512:(dc + 1) * 512])
                        for i_ in range(4):
                            nc.tensor.matmul(pss_op[(tt, 0)][:], identr[:],
                                             warm_rhs[:],
                                             start=True, stop=True)
                        stats = p3s.tile([128, 2, 6], F32, tag="stats")
                        for g in range(2):
                            nc.vector.bn_stats(
                                stats[:, g, :],
                                y_sb[:, tt, g * 512:(g + 1) * 512])
                        mv = p3s.tile([128, 2], F32, tag="mv")
                        nc.vector.bn_aggr(mv[:], stats[:])
                        sd = p3s.tile([128, 1], F32, tag="sd")
                        nc.scalar.activation(sd[:], mv[:, 1:2], AF.Sqrt,
                                             bias=eps[:], scale=1.0)
                        rstd = p3s.tile([128, 1], F32, tag="rstd")
                        nc.vector.reciprocal(rstd[:], sd[:])
                        nb2 = p3s.tile([128, 1], F32, tag="nb2")
                        nc.vector.scalar_tensor_tensor(
                            nb2[:], mv[:, 0:1], -1.0, rstd[:],
                            MUL, MUL)
                        nc.scalar.activation(hns[:, tt, :], y_sb[:, tt, :],
                                             AF.Identity,
                                             bias=nb2[:], scale=rstd[:])

                # transpose h_norm -> hT8 + hTlo (hi/lo fp8 split: h ~
                # hT8 + hTlo to bf16-class accuracy), then the MLP
                ln_mlp = tc.tile_pool(name="p3acc", bufs=3, space="PSUM")
                p3acc = ln_mlp.__enter__()
                p3pt_ctx = tc.tile_pool(name="p3pt", bufs=2, space="PSUM")
                p3pt = p3pt_ctx.__enter__()
                # prefetch the first mlp-in weight tiles during LN
                wis = {}
                for ff in range(8):
                    wis[ff] = mlpw.tile([128, 2, 8, 128], E4, tag="wi",
                                        name=f"wi{ff}")
                    nc.sync.dma_start(wis[ff][:], w8i_io[:, ff, :, :, :])
                # dc-major: the first mlp-in matmul contracts dc blocks 0-1
                # over ALL tokens, so this order lets the MLP start as soon
                # as the first two dc columns are transposed
                wtm = p3acc.tile([128, 512], F32, tag="acc", name="warmmlp")
                for dc in range(8):
                    for i_ in range(3):   # keep the PE p-state up
                        nc.tensor.matmul(wtm[:], identr[:], warm_rhs[:],
                                         start=True, stop=True)
                    for tt in range(4):
                        pt = p3pt.tile([128, 128], BF, tag="pt3")
                        nc.tensor.transpose(
                            pt[:], hns[:, tt, dc * 128:(dc + 1) * 128],
                            identb[:])
                        tsl = slice(tt * 128, (tt + 1) * 128)
                        # split the psum->fp8 copies across Act and DVE
                        if tt % 2 == 0:
                            nc.scalar.copy(hT8[:, dc, tsl], pt[:])
                        else:
                            nc.vector.tensor_copy(hT8[:, dc, tsl], pt[:])

                # transpose pool freed -> 6-buf mlp-out accumulators so
                # the dc=1 group starts while dc=0 drains
                p3pt_ctx.__exit__(None, None, None)
                p3mo_ctx = tc.tile_pool(name="p3mo", bufs=5, space="PSUM")
                p3mo = p3mo_ctx.__enter__()

                # MLP in + gelu, 3-pass hi/lo DR
                hmT8 = p3sb.tile([128, 32, 512], E4, tag="hmT8")
                hmTlo = p3sb.tile([128, 32, 512], E4, tag="hmTlo")
                for ff in range(32):
                    if ff in wis:
                        wi = wis[ff]
                    else:
                        wi = mlpw.tile([128, 2, 8, 128], E4, tag="wi",
                                       name=f"wi{ff}")
                        nc.sync.dma_start(wi[:], w8i_io[:, ff, :, :, :])
                    ps = p3acc.tile([128, 512], F32, tag="acc")
                    # 2-pass: h8 @ (Wh + Wl) - W at bf16-class accuracy, h
                    # at fp8 (errstudy2 V4: 1.2e-2 vs the 2e-2 gate)
                    first, last = (0, 0), (1, 3)
                    for pi, (wsl, h) in enumerate(
                            ((0, hT8), (1, hT8))):
                        for pp in range(4):
                            nc.tensor.matmul(
                                ps[:], wi[:, wsl, 2 * pp:2 * pp + 2, :],
                                h[:, 2 * pp:2 * pp + 2, :],
                                start=((pi, pp) == first),
                                stop=((pi, pp) == last), perf_mode=DR)
                        if ff == 0:
                            # fill PE idle while transposes/subs complete
                            for i_ in range(3):
                                nc.tensor.matmul(wtm[:], identr[:],
                                                 warm_rhs[:],
                                                 start=True, stop=True)
                    t2 = p3r.tile([128, 512], BF, tag="t2",
                                  name=f"t2_{ff}")
                    nc.scalar.activation(t2[:], ps[:], AF.Gelu,
                                         bias=b_in[:, ff:ff + 1],
                                         scale=1.0 / WS)
                    # hi/lo fp8 split of the gelu output for the DR mlp_out
                    nc.vector.tensor_copy(hmT8[:, ff, :], t2[:])
                    if ff % 2 == 0:
                        nc.vector.tensor_sub(hmTlo[:, ff, :], t2[:],
                                             hmT8[:, ff, :])
                    else:
                        nc.gpsimd.tensor_sub(hmTlo[:, ff, :], t2[:],
                                             hmT8[:, ff, :])

                # MLP out + final residual: 3-pass hi/lo DR over ff-pairs
                out_r = out_io.rearrange("(t p) d -> p t d", p=128)
                for dc in range(2):
                    pss = [p3mo.tile([128, 512], F32, tag="mo",
                                     name=f"mo{dc}_{i}") for i in range(4)]
                    for ffp in range(16):
                        w2 = mlpw.tile([128, 4, 512], E4, tag="w2",
                                       name=f"w2_{dc}_{ffp}")
                        nc.sync.dma_start(
                            w2[:],
                            bass.AP(tensor=w8o_t,
                                    offset=(ffp * 128 * 4 * D + dc * 512),
                                    ap=[[4 * D, 128], [D, 4], [1, 512]]))
                        for tt in range(4):
                            tsl = slice(tt * 128, (tt + 1) * 128)
                            for pi, (g, wsl) in enumerate(
                                    ((hmT8, 0), (hmT8, 1), (hmTlo, 0))):
                                nc.tensor.matmul(
                                    pss[tt][:],
                                    g[:, 2 * ffp:2 * ffp + 2, tsl],
                                    w2[:, 2 * wsl:2 * wsl + 2, :],
                                    start=(ffp == 0 and pi == 0),
                                    stop=False,
                                    perf_mode=DR)
                    for tt in range(4):
                        # inject y2 = WS*(y + b_out); psum carries x32 from
                        # the weight prescale, so one Act scaled copy
                        # finishes the kernel output
                        nc.tensor.matmul(
                            pss[tt][:], identr32[:],
                            y2_sb[:, tt, dc * 512:(dc + 1) * 512],
                            start=False, stop=True)
                        fin = p3s.tile([128, 512], F32, tag="fin")
                        nc.scalar.activation(fin[:], pss[tt][:], AF.Copy,
                                             scale=1.0 / WS)
                        nc.sync.dma_start(
                            out_r[:, tt, dc * 512:(dc + 1) * 512], fin[:])
                for ctx in (p3mo_ctx, ln_mlp):
                    ctx.__exit__(None, None, None)

    nc.compile()
    return nc


def _host_prep(x, alibi, ln1_w, w_qkv, w_out, ln2_w, w_mlp_in, b_mlp_in,
               w_mlp_out, b_mlp_out):
    f32 = np.float32
    x = np.asarray(x, f32)
    x_flat = np.ascontiguousarray(x.reshape(NTOK, D))
    w_qkv = np.asarray(w_qkv, f32)
    w_out = np.asarray(w_out, f32)
    w_mlp_in = np.asarray(w_mlp_in, f32)
    w_mlp_out = np.asarray(w_mlp_out, f32)
    b_mlp_in = np.asarray(b_mlp_in, f32)
    b_mlp_out = np.asarray(b_mlp_out, f32)
    ln2_w = np.asarray(ln2_w, f32)
    alibi = np.asarray(alibi, f32)

    # x8 [p, pp, s, tok] = x[tok, (pp*2+s)*128 + p]
    x8 = np.ascontiguousarray(
        x_flat.T.reshape(4, 2, 128, NTOK).transpose(2, 0, 1, 3)).astype(E4M3)

    # identities
    i240 = (IDENT_V * np.eye(128, dtype=f32)).astype(E4M3)
    identb = np.eye(128, dtype=f32).astype(BF16)

    # out-proj weights: wo8[p, pp, s, dcol] = WS * w_out[dcol, (pp*2+s)*128+p]
    wo8 = np.ascontiguousarray(
        (WS * w_out).T.reshape(4, 2, 128, D).transpose(2, 0, 1, 3)
    ).astype(E4M3)

    w_in_eff = WS * w_mlp_in * ln2_w[None, :]     # (FF, D), x32 for fp8
    w_inP = np.ascontiguousarray(
        w_in_eff.reshape(32, 128, 8, 128).transpose(3, 0, 2, 1))
    w8i_h = w_inP.astype(E4M3)
    w8i_l = (w_inP - w8i_h.astype(np.float32)).astype(E4M3)
    # packed hi/lo: [p, ff, hl, kk, fin]
    w8i = np.ascontiguousarray(
        np.stack([w8i_h, w8i_l], axis=2))
    # w2P [ffp, p, s, dcol] = (32*w_mlp_out)[dcol, (ffp*2+s)*128+p]
    w2P = np.ascontiguousarray(
        (WS * w_mlp_out).T.reshape(16, 2, 128, D).transpose(0, 2, 1, 3))
    w8o_h = w2P.astype(E4M3)
    w8o_l = (w2P - w8o_h.astype(np.float32)).astype(E4M3)
    # packed hi/lo: [ffp, p, hl, s, dcol]
    w8o = np.ascontiguousarray(np.stack([w8o_h, w8o_l], axis=2))
    b_inT = np.ascontiguousarray(b_mlp_in.reshape(32, 128).T)

    in_maps = []
    for c in range(NCORES):
        h0 = HPC * c
        qrows = WS * w_qkv[h0 * Dh:(h0 + HPC) * Dh]
        krows = WS * w_qkv[H * Dh + h0 * Dh:H * Dh + (h0 + HPC) * Dh]
        vrows = WS * w_qkv[2 * H * Dh + h0 * Dh:2 * H * Dh + (h0 + HPC) * Dh]
        wq = np.concatenate([qrows, krows, vrows], 0)   # [384, D]
        # wq8 [p, pp, s, mcol] = wq[mcol, (pp*2+s)*128 + p]
        wq8 = np.ascontiguousarray(
            wq.T.reshape(4, 2, 128, 384).transpose(2, 0, 1, 3)).astype(E4M3)
        # alibi transposed to [hl, k, q]; k-row blocks gathered per engine
        # assignment (al8 row-block i = ACT_KTS[i], ab16 row-block i =
        # DVE_KTS[i])
        alc = alibi[0, h0:h0 + HPC].transpose(0, 2, 1)  # [HPC, T(k), T(q)]
        alc_kt = alc.reshape(HPC, KT, 128, T)
        al8 = np.ascontiguousarray(
            AL_SCALE * alc_kt[:, ACT_KTS].reshape(HPC, NAK * 128, T)
        ).astype(E4M3)
        ab16 = np.ascontiguousarray(
            LOG2E8 * alc_kt[:, DVE_KTS].reshape(HPC, NBK * 128, T) + BT_BIAS
        ).astype(np.float16)
        # residual pre-scaled x32: it is injected into the x32-scaled
        # out-proj / mlp-out PSUMs via f32r identity matmuls
        x_res = np.ascontiguousarray(WS * x_flat[c * CHUNK:(c + 1) * CHUNK])
        in_maps.append({
            "x8": x8, "wq8": wq8, "al8": al8, "ab16": ab16, "i240": i240,
            "identb": identb, "wo8": wo8,
            "x_res": x_res, "b_outT": b_mlp_out.reshape(1, D),
            "w8i": w8i, "b_inT": b_inT, "w8o": w8o,
        })
    return in_maps


def _get_compiled():
    global _COMPILED
    if _COMPILED is None:
        _COMPILED = _build()
    return _COMPILED


def kernel(_trace=False, **inputs):
    nc = _get_compiled()
    in_maps = _host_prep(**inputs)
    res = None
    for attempt in range(3):
        try:
            res = run_bass_kernel_spmd(nc, in_maps,
                                       core_ids=list(range(NCORES)),
                                       trace=_trace)
            break
        except Exception:
            if attempt == 2:
                raise
    out = np.concatenate([res.results[c]["out"] for c in range(NCORES)], 0)
    out = out.reshape(B, T, D).astype(np.float32)
    if _trace:
        return out, res
    return out


# revision 69
# speedup vs baseline: 1.2609x; 1.0101x over previous
"""Trainium2 Bass kernel for nn_AttentionBlock (B=2, T=2048, D=1024, H=16,
Dh=64, Ff=4096), SPMD across 8 NeuronCores in one NEFF launch.

Sharding:
  - Phase 1+2 (QKV projection + attention): 2 heads per core over all 4096
    tokens. Phase 3 (out-proj + residual + LayerNorm + MLP): 512 tokens per
    core, re-sharded via an AllToAll of the attention output (fp8, 0.5 MiB).

Numerics (validated end-to-end in numpy against the reference, see
errstudy2.py: ~2e-3 max-rel vs the 2e-2 gate):
  - Attention matmuls run fp8e4m3 DoubleRow as before (QKV projection,
    Q.K^T scores, attn@V, output projection). Weights pre-scaled x32.
  - The softmax exp is split across two engines per k-tile (the Act engine
    was the attention bottleneck at ~1us per [128,1024] exp):
      * kt < NAK: alibi injected into the scores PSUM by a plain fp8 matmul
        against a 240-valued identity (as before), then Act exp ->
        fp8 attention weights.
      * kt >= NAK: DVE computes the fp8 weights directly via the
        exp bit-trick: i8 = psum*(8*log2e/8192) + (alibi*8*log2e + 56 [+rnd])
        written as int8 and bit-viewed as e4m3 ~= exp(s+alibi). The f16
        alibi-bias tile fuses the alibi add, so no PE inject is needed.
    The bit-trick's mantissa-interpolation error (~3%) is the same class as
    the e4m3 quantization the Act path already incurs; softmax averaging
    washes both out (errstudy2: 2.01e-3 vs 2.13e-3).
  - Scores DoubleRow uses a zeroed second k-tile slot; attn@V pairs adjacent
    score k-tiles. V padded to 128 cols (col 64 = softmax-denominator ones
    column = 32).
  - mlp_in runs 2-pass (h8 @ (W_hi + W_lo): weights at bf16-class accuracy,
    activations fp8); mlp_out stays 3-pass hi/lo. End-to-end rel_absmax
    measured 1.204e-2 on HW (errstudy2 V4 predicted 1.206e-2) vs the 2e-2
    gate. The A2A is split by head-half so the first collective overlaps
    the second half of attention, and the out-projection runs in two
    partition-half stages with the residual injected via f32r identity
    matmuls (readouts become pure Act/DVE scaled copies).

kernel(**inputs) takes FULL unsharded inputs, returns the FULL output.
"""

import sys

for _p in ("/opt/trn_rl_repo", "/root/.axon_site/_ro/trn_rl_repo"):
    if _p not in sys.path:
        sys.path.insert(0, _p)

import numpy as np
import ml_dtypes

import concourse.bass as bass
import concourse.tile as tile
from concourse import bacc, mybir
from concourse.bass_utils import run_bass_kernel_spmd

BF16 = ml_dtypes.bfloat16
E4M3 = ml_dtypes.float8_e4m3   # TRN float8e4: max 240

B, T, D, H, Dh, FF = 2, 2048, 1024, 16, 64, 4096
NTOK = B * T            # 4096
NCORES = 8
CHUNK = NTOK // NCORES  # 512 tokens per core
HPC = H // NCORES       # 2 heads per core
KT = T // 128           # 16 k-tiles per batch

# Interleaved engine assignment of the 16 k-tiles per pass: Act tiles get
# true exp (+PE alibi inject), DVE tiles get the fused bit-trick exp.
# Interleaving keeps both engines fed as the PE emits scores in kt order.
ACT_KTS = (0, 1, 3, 4, 6, 8, 10, 12, 14, 15)
DVE_KTS = (2, 5, 7, 9, 11, 13)
NAK = len(ACT_KTS)
NBK = len(DVE_KTS)
A_POS = {kt: i for i, kt in enumerate(ACT_KTS)}
D_POS = {kt: i for i, kt in enumerate(DVE_KTS)}

WS = 32.0               # host weight pre-scale for fp8 range
EXP_SCALE = 1.0 / (WS * WS * 8.0)   # 1/8192: undo q,k x32 and apply 1/sqrt(Dh)
IDENT_V = 240.0                      # e4m3-exact identity magnitude
AL_SCALE = (WS * WS * 8.0) / IDENT_V  # alibi pre-scale so PSUM alibi = 8192*a
LOG2E8 = float(8.0 * np.log2(np.e))  # e4m3 bit-trick exp slope
BT_SCALE = LOG2E8 / 8192.0
BT_BIAS = 56.0          # e4m3 exponent-bias term (see _host_prep; rounding
                        # offset calibrated on HW via probe.py)

F32 = mybir.dt.float32
F32R = mybir.dt.float32r
BF = mybir.dt.bfloat16
E4 = mybir.dt.float8e4
F16 = mybir.dt.float16
I8 = mybir.dt.int8
AF = mybir.ActivationFunctionType
DR = mybir.MatmulPerfMode.DoubleRow
MUL = mybir.AluOpType.mult
ADD = mybir.AluOpType.add

_COMPILED = None


def _build(sim1=False):
    nc = bacc.Bacc("TRN2", target_bir_lowering=False, debug=False,
                   num_devices=1 if sim1 else NCORES)

    # ---- kernel I/O (per core) ----
    # x packed [p, pp, s, tok] = x[tok, (pp*2+s)*128 + p], fp8
    x8_io = nc.dram_tensor("x8", [128, 4, 2, NTOK], E4, kind="ExternalInput").ap()
    # qkv weights x32 packed [p, pp, s, m*128+j] = w[row(m,j), (pp*2+s)*128+p]
    wq8_io = nc.dram_tensor("wq8", [128, 4, 2, 384], E4, kind="ExternalInput").ap()
    # alibi*AL_SCALE transposed, Act k-tiles only: al8[hl, k, q], k < NAK*128
    al8_t = nc.dram_tensor("al8", [HPC, NAK * 128, T], E4, kind="ExternalInput")
    # bit-trick alibi bias, f16: ab16[hl, k - NAK*128, q]
    ab16_t = nc.dram_tensor("ab16", [HPC, NBK * 128, T], F16,
                            kind="ExternalInput")
    # identity of value IDENT_V for the alibi inject matmul
    i240_io = nc.dram_tensor("i240", [128, 128], E4, kind="ExternalInput").ap()
    identb_io = nc.dram_tensor("identb", [128, 128], BF, kind="ExternalInput").ap()
    # out-proj weights x32 packed [p, pp, s, dcol] = WS*w_out[dcol, (pp*2+s)*128+p]
    wo8_io = nc.dram_tensor("wo8", [128, 4, 2, D], E4, kind="ExternalInput").ap()
    x_res_io = nc.dram_tensor("x_res", [CHUNK, D], F32, kind="ExternalInput").ap()
    b_outT_io = nc.dram_tensor("b_outT", [1, D], F32, kind="ExternalInput").ap()
    # mlp-in weights, hi/lo packed in one tensor:
    # [p, ff, hl, kk, fin] = hi/lo of (32*w_in_eff)[ff*128+fin, kk*128+p]
    w8i_io = nc.dram_tensor("w8i", [128, 32, 2, 8, 128], E4,
                            kind="ExternalInput").ap()
    b_inT_io = nc.dram_tensor("b_inT", [128, 32], F32, kind="ExternalInput").ap()
    # mlp-out weights, hi/lo packed: [ffp, p, hl, s, dcol] =
    # hi/lo of (32*w_mlp_out)[dcol, (ffp*2+s)*128+p]
    w8o_t = nc.dram_tensor("w8o", [16, 128, 2, 2, D], E4,
                           kind="ExternalInput")
    out_io = nc.dram_tensor("out", [CHUNK, D], F32, kind="ExternalOutput").ap()

    # ---- internal DRAM ----
    # per-head-half collective buffers: A2A#0 fires after the hl=0 passes
    # and overlaps the hl=1 attention; A2A#1 covers the rest.
    cc_send = [nc.dram_tensor(f"cc_send{hl}", [D // 2, CHUNK], E4)
               for hl in range(2)]
    cc_recv = [nc.dram_tensor(f"cc_recv{hl}", [D // 2, CHUNK], E4)
               for hl in range(2)]

    with tile.TileContext(nc) as tc:
        with tc.tile_pool(name="consts", bufs=1) as consts:
            i240 = consts.tile([128, 128], E4, tag="i240")
            nc.sync.dma_start(i240[:], i240_io)
            identb = consts.tile([128, 128], BF, tag="identb")
            nc.sync.dma_start(identb[:], identb_io)
            warm_f = consts.tile([128, 512], F32, tag="warm_f")
            nc.vector.memset(warm_f[:], 0.5)
            warm_rhs = consts.tile([128, 512], F32R, tag="warm_rhs")
            nc.vector.tensor_copy(warm_rhs[:], warm_f[:])
            identr = consts.tile([128, 128], F32R, tag="identr")
            nc.vector.tensor_copy(identr[:], identb[:])
            # 32-scaled f32r identity: injects the (unscaled) y2 residual
            # into the x32-scaled mlp-out PSUM
            identr32 = consts.tile([128, 128], F32R, tag="identr32")
            nc.vector.tensor_scalar(identr32[:], identb[:], WS, None, MUL)
            # phase-3 input tiles (loaded mid-attention; consts pool lives
            # for the whole kernel)
            wo8 = consts.tile([128, 4, 2, D], E4, tag="wo8")
            b_in = consts.tile([128, 32], F32, tag="b_in")
            bob = consts.tile([1, D], F32, tag="bob")
            xrs = consts.tile([128, 4, D], F32, tag="xr")
            # f32r copy for the residual-inject matmul (fp32r operands must
            # be produced rounded; idle-Pool copies during attention)
            xrsr = consts.tile([128, 4, D], F32R, tag="xrr")
            # re-sharded attention output, assembled per head-half as each
            # AllToAll lands (partitions 0-63 <- hl=0, 64-127 <- hl=1)
            yrT = consts.tile([128, 8, 512], E4, tag="yrT")

            with tc.tile_pool(name="qkv", bufs=1) as qkv, \
                 tc.tile_pool(name="alb", bufs=3) as albp, \
                 tc.tile_pool(name="abb", bufs=3) as abbp:
                al_cache = {}

                def load_alibi(hl, qc, ring, splits=1):
                    # ring choice controls WHEN the transfer runs relative
                    # to the x8 loads on the shared DMA engines. splits>1
                    # halves the first pass's wait: its early k-tiles'
                    # alibi lands while the rest still streams.
                    alt = albp.tile([128, NAK, 1024], E4, tag="al",
                                    name=f"al{hl}_{qc}")
                    abt = abbp.tile([128, NBK, 1024], F16, tag="ab",
                                    name=f"ab{hl}_{qc}")
                    cuts_a = [NAK * i // splits for i in range(splits + 1)]
                    cuts_b = [NBK * i // splits for i in range(splits + 1)]
                    for s in range(splits):
                        a0, a1 = cuts_a[s], cuts_a[s + 1]
                        ring.dma_start(
                            alt[:, a0:a1, :],
                            bass.AP(tensor=al8_t,
                                    offset=(hl * NAK + a0) * 128 * T
                                    + qc * 1024,
                                    ap=[[T, 128], [128 * T, a1 - a0],
                                        [1, 1024]]))
                        b0_, b1_ = cuts_b[s], cuts_b[s + 1]
                        ring.dma_start(
                            abt[:, b0_:b1_, :],
                            bass.AP(tensor=ab16_t,
                                    offset=(hl * NBK + b0_) * 128 * T
                                    + qc * 1024,
                                    ap=[[T, 128], [128 * T, b1_ - b0_],
                                        [1, 1024]]))
                    al_cache[(hl, qc)] = (alt, abt)

                q8s, k8s, v8s = [], [], []
                for b in range(2):
                    q8 = qkv.tile([128, 2, T], E4, tag=f"q8{b}", name=f"q8{b}")
                    k8 = qkv.tile([128, 2, T], E4, tag=f"k8{b}", name=f"k8{b}")
                    # v8 [p, ktpair, slot, hl, 128]: cols 0-63 v, col 64 = WS
                    # (softmax denominator), cols 65-127 zero (DR stationary
                    # width must be 32/64/128); zeroed first - the v8 copies
                    # need them before the q8/k8 zero-slots are read
                    v8 = qkv.tile([128, 8, 2, 2, 128], E4, tag=f"v8{b}",
                                  name=f"v8{b}")
                    nc.gpsimd.memset(v8[:, :, :, :, 64:], 0.0)
                    nc.vector.memset(v8[:, :, :, :, 64:65], WS)
                    q8s.append(q8); k8s.append(k8); v8s.append(v8)
                for b in range(2):
                    # zero-slot for scores DR second k-tile
                    nc.gpsimd.memset(k8s[b][:, 1, :], 0.0)
                    nc.gpsimd.memset(q8s[b][:, 1, :], 0.0)

                # ---------------- phase 1: QKV projection ----------------
                with tc.tile_pool(name="p1x", bufs=2) as p1x, \
                     tc.tile_pool(name="p1w", bufs=1) as p1w, \
                     tc.tile_pool(name="p1t", bufs=3) as p1t, \
                     tc.tile_pool(name="p1ps", bufs=6, space="PSUM") as p1ps, \
                     tc.tile_pool(name="p1pt", bufs=2, space="PSUM") as p1pt:
                    wq8 = p1w.tile([128, 4, 2, 384], E4, tag="wq8")
                    nc.sync.dma_start(wq8[:], wq8_io)
                    for wi_ in range(4):
                        wps = p1pt.tile([128, 512], F32, tag="pt",
                                        name=f"warms{wi_}")
                        nc.tensor.matmul(wps[:], identr[:], warm_rhs[:],
                                         start=True, stop=True)

                    # x loads first (proj critical path), alibi loads woven
                    # in between so they land before their passes without
                    # delaying the projection on the shared DMA engines
                    xtss = []
                    for b in range(2):
                        xts = p1x.tile([128, 4, 2, 2048], E4, tag="xt",
                                       name=f"xt{b}")
                        xtss.append(xts)
                        for cc4 in range(4):
                            nc.sync.dma_start(
                                xts[:, :, :, cc4 * 512:(cc4 + 1) * 512],
                                x8_io[:, :, :,
                                      b * 2048 + cc4 * 512:
                                      b * 2048 + (cc4 + 1) * 512])
                    load_alibi(0, 0, nc.sync, splits=3)
                    load_alibi(0, 1, nc.sync)

                    for b in range(2):
                        q8, k8, v8 = q8s[b], k8s[b], v8s[b]
                        xts = xtss[b]
                        with nc.named_scope(f"qkvproj{b}"):
                            for t in range(4):
                                tsl = slice(t * 512, (t + 1) * 512)
                                for m in range(3):   # q, k, v
                                    ps = p1ps.tile([128, 512], F32, tag="proj",
                                                   name=f"proj{b}_{t}_{m}")
                                    for pp in range(4):
                                        nc.tensor.matmul(
                                            ps[:],
                                            wq8[:, pp, :, m * 128:(m + 1) * 128],
                                            xts[:, pp, :, tsl],
                                            start=(pp == 0), stop=(pp == 3),
                                            perf_mode=DR)
                                    if m == 0:
                                        nc.scalar.copy(q8[:, 0, tsl], ps[:])
                                    elif m == 1:
                                        nc.vector.tensor_copy(k8[:, 0, tsl],
                                                              ps[:])
                                    else:
                                        vt = p1t.tile([128, 512], BF, tag="vt",
                                                      name=f"vt{b}_{t}")
                                        nc.scalar.copy(vt[:], ps[:])
                                        for j in range(4):
                                            ti = t * 4 + j
                                            pt = p1pt.tile([128, 128], BF,
                                                           tag="pt",
                                                           name=f"pt{b}_{ti}")
                                            nc.tensor.transpose(
                                                pt[:],
                                                vt[:, j * 128:(j + 1) * 128],
                                                identb[:])
                                            nc.vector.tensor_copy(
                                                v8[:, ti // 2, ti % 2, :, 0:64],
                                                pt[:].rearrange(
                                                    "p (h d) -> p h d", h=2))
                        if b == 1:
                            # sync ring: the pool rotation (bufs=3) holds
                            # the 4th tile until pass (0,0,1) completes, so
                            # these transfers naturally run mid-attention
                            # without competing with the x8/al(0,*) loads;
                            # nothing urgent sits behind them on SP
                            load_alibi(1, 0, nc.sync)
                            load_alibi(1, 1, nc.sync)

                # ---------------- phase 2: attention ----------------
                with nc.named_scope("attn"), \
                     tc.tile_pool(name="exps", bufs=8) as expp, \
                     tc.tile_pool(name="sps", bufs=3, space="PSUM") as spsp, \
                     tc.tile_pool(name="yups", bufs=1, space="PSUM") as yupp, \
                     tc.tile_pool(name="nrm", bufs=2) as nrmp, \
                     tc.tile_pool(name="yns", bufs=4) as ynp:
                    # each pass's final yn multiply is emitted mid-NEXT-pass
                    # so the in-order DVE queue never stalls on the Pool
                    # broadcast it depends on
                    pending_norm = []

                    def flush_norm():
                        while pending_norm:
                            pending_norm.pop(0)()

                    def attn_pass(hl, qc, b):
                        alt, abt = al_cache[(hl, qc)]
                        hsl = slice(hl * 64, (hl + 1) * 64)
                        yu = yupp.tile([128, 1024], F32, tag="yu",
                                       name=f"yu{hl}_{qc}_{b}")
                        ex8 = None
                        exq = []  # deferred attnV ops: (ktp, ex8)
                        for kt in range(KT):
                            is_act = kt in A_POS
                            sp = spsp.tile([128, 1024], F32, tag="sp",
                                           name=f"sp{hl}_{qc}_{b}_{kt}")
                            for qh in range(2):
                                qsl = slice(qc * 1024 + qh * 512,
                                            qc * 1024 + (qh + 1) * 512)
                                osl = slice(qh * 512, (qh + 1) * 512)
                                nc.tensor.matmul(
                                    sp[:, osl],
                                    k8s[b][hsl, :, kt * 128:(kt + 1) * 128],
                                    q8s[b][hsl, :, qsl],
                                    start=True, stop=not is_act,
                                    perf_mode=DR)
                                if is_act:
                                    nc.tensor.matmul(
                                        sp[:, osl],
                                        i240[:],
                                        alt[:, A_POS[kt], osl],
                                        start=False, stop=True)
                            if kt % 2 == 0:
                                ex8 = expp.tile([128, 2, 1024], E4, tag="ex",
                                                name=f"ex{hl}_{qc}_{b}_{kt}")
                            if is_act:
                                nc.scalar.activation(ex8[:, kt % 2, :], sp[:],
                                                     AF.Exp, scale=EXP_SCALE)
                            else:
                                nc.vector.scalar_tensor_tensor(
                                    ex8[:, kt % 2, :].bitcast(I8), sp[:],
                                    BT_SCALE, abt[:, D_POS[kt], :], MUL, ADD)
                            if kt == 5:
                                # previous pass's deferred yn multiply: by
                                # now its Pool broadcast has finished, so
                                # the DVE never blocks on it
                                flush_norm()
                            if kt % 2 == 1:
                                exq.append((kt // 2, ex8))
                            # drain attnV four k-tile-pairs behind the scores
                            # so the PE queue never waits on a fresh exp nor
                            # on the previous pass's norm freeing yu
                            while exq and (exq[0][0] <= kt // 2 - 4
                                           or kt == KT - 1):
                                ktp, exd = exq.pop(0)
                                for qh in range(2):
                                    osl = slice(qh * 512, (qh + 1) * 512)
                                    nc.tensor.matmul(
                                        yu[:, osl],
                                        v8s[b][:, ktp, :, hl, :],
                                        exd[:, :, osl],
                                        start=(ktp == 0), stop=(ktp == 7),
                                        perf_mode=DR)
                        rec = nrmp.tile([1, 1024], F32, tag="rec",
                                        name=f"rec{hl}_{qc}_{b}")
                        nc.vector.reciprocal(rec[:], yu[64:65, :])
                        bc = nrmp.tile([64, 1024], F32, tag="bc",
                                       name=f"bc{hl}_{qc}_{b}")
                        nc.gpsimd.partition_broadcast(bc[:], rec[:])

                        def norm_fin(hl=hl, qc=qc, b=b, yu=yu, bc=bc):
                            i = b * 2 + qc
                            yn = ynp.tile([64, 1024], E4, tag="yn",
                                          name=f"yn{hl}_{qc}_{b}")
                            nc.vector.tensor_mul(yn[:], yu[0:64, :], bc[:])
                            nc.sync.dma_start(
                                bass.AP(tensor=cc_send[hl],
                                        offset=2 * i * 64 * 512,
                                        ap=[[512, 64], [64 * 512, 2],
                                            [1, 512]]),
                                yn[:].rearrange("p (h c) -> p h c", h=2))
                        pending_norm.append(norm_fin)

                    def run_a2a(hl):
                        with nc.named_scope(f"a2a{hl}"):
                            if sim1:
                                nc.sync.dma_start(cc_recv[hl][:],
                                                  cc_send[hl][:])
                            else:
                                nc.gpsimd.collective_compute(
                                    "AllToAll", mybir.AluOpType.bypass,
                                    replica_groups=[list(range(NCORES))],
                                    ins=[cc_send[hl][:]],
                                    outs=[cc_recv[hl][:]])
                            # assemble the hl half of yrT: partitions
                            # hl*64.. <- recv blocks [64, 512] per src core
                            # (sync ring: a scalar-ring DGE would block the
                            # Act queue's exps until the collective lands)
                            nc.sync.dma_start(
                                yrT[hl * 64:(hl + 1) * 64, :, :],
                                bass.AP(tensor=cc_recv[hl], offset=0,
                                        ap=[[512, 64], [64 * 512, 8],
                                            [1, 512]]))

                    def pe_keepwarm(n, pool, tag, name):
                        # bridge PE idle gaps so the p-state never drops
                        wt = pool.tile([128, 1024], F32, tag=tag, name=name)
                        for i_ in range(n):
                            nc.tensor.matmul(wt[:, 0:512], identr[:],
                                             warm_rhs[:],
                                             start=True, stop=True)

                    first = True
                    for hl in range(2):
                        for qc in range(2):
                            for b in range(2):
                                attn_pass(hl, qc, b)
                                if first:
                                    # no-dependency phase-3 loads: issued here
                                    # so they transfer during attention
                                    nc.sync.dma_start(wo8[:], wo8_io[:])
                                    nc.sync.dma_start(b_in[:], b_inT_io[:])
                                    nc.sync.dma_start(bob[:], b_outT_io[:])
                                    x_res_r = x_res_io.rearrange(
                                        "(t p) d -> p t d", p=128)
                                    for tt_ in range(4):
                                        nc.sync.dma_start(xrs[:, tt_, :],
                                                          x_res_r[:, tt_, :])
                                        nc.gpsimd.tensor_copy(
                                            xrsr[:, tt_, :], xrs[:, tt_, :])
                                    first = False
                        if hl == 0:
                            # A2A#0 emission rides the deferred-norm queue:
                            # it lands right after pass 3's yn DMA, inside
                            # pass 4's DVE stream (no hl-boundary stall)
                            pending_norm.append(lambda: run_a2a(0))
                        else:
                            pe_keepwarm(13, spsp, "sp", "warm_a2a")
                            flush_norm()
                            run_a2a(1)

            # ---------------- phase 3: out-proj + LN + MLP ----------------
            with nc.named_scope("mlp"), \
                 tc.tile_pool(name="p3sb", bufs=1) as p3sb, \
                 tc.tile_pool(name="p3r", bufs=3) as p3r, \
                 tc.tile_pool(name="p3s", bufs=4) as p3s, \
                 tc.tile_pool(name="mlpw", bufs=8) as mlpw:
                # dummy Sqrt during the A2A hole: pulls the Act
                # function-table swap off the preamble critical path
                dum = p3s.tile([128, 1], F32, tag="dum")
                nc.scalar.activation(dum[:], warm_f[:, 0:1], AF.Sqrt,
                                     scale=1.0)
                bb = p3sb.tile([128, D], F32, tag="bb")
                nc.gpsimd.partition_broadcast(bb[:], bob[:])

                y_sb = p3sb.tile([128, 4, D], F32, tag="y_sb")
                # f32r: written rounded by the Pool stt, consumed by the
                # final-residual inject matmul
                y2_sb = p3sb.tile([128, 4, D], F32R, tag="y2_sb")
                hns = p3sb.tile([128, 4, D], BF, tag="hns")
                hT8 = p3sb.tile([128, 8, 512], E4, tag="hT8")
                eps = p3sb.tile([128, 1], F32, tag="eps")
                nc.vector.memset(eps[:], 1e-5)
                # 2-stage out-proj: the hl=0 partials (partitions 0-63 of
                # each pp slot) run as soon as A2A#0 lands, overlapping the
                # A2A#1 transfer; stage 2 accumulates the hl=1 rows and
                # injects the x32-scaled residual via an f32r identity
                # matmul, so the readout is a pure scaled copy.
                with tc.tile_pool(name="p3acc8", bufs=8,
                                  space="PSUM") as p3acc8:
                    pss_op = {}
                    for hstage in range(2):
                        psl = slice(hstage * 64, (hstage + 1) * 64)
                        for tt in range(4):
                            for dc in range(2):
                                if hstage == 0:
                                    pss_op[(tt, dc)] = p3acc8.tile(
                                        [128, 512], F32, tag="acc",
                                        name=f"acc{tt}_{dc}")
                                ps = pss_op[(tt, dc)]
                                for pp in range(4):
                                    nc.tensor.matmul(
                                        ps[:],
                                        yrT[psl, 2 * pp:2 * pp + 2,
                                            tt * 128:(tt + 1) * 128],
                                        wo8[psl, pp, :,
                                            dc * 512:(dc + 1) * 512],
                                        start=(hstage == 0 and pp == 0),
                                        stop=False,
                                        perf_mode=DR)
                                if hstage == 1:
                                    nc.tensor.matmul(
                                        ps[:], identr[:],
                                        xrsr[:, tt, dc * 512:(dc + 1) * 512],
                                        start=False, stop=True)
                    # readouts interleaved with the LN chains (dc0 on Act,
                    # dc1 on DVE) so the in-order engine queues dovetail;
                    # warm matmuls into just-read accumulators keep the PE
                    # p-state up through this window
                    for tt in range(4):
                        nc.scalar.activation(
                            y_sb[:, tt, 0:512], pss_op[(tt, 0)][:],
                            AF.Copy, scale=1.0 / WS)
                        nc.vector.tensor_scalar(
                            y_sb[:, tt, 512:1024], pss_op[(tt, 1)][:],
                            1.0 / WS, None, MUL)
                        for dc in range(2):
                            # y2 = y + b_mlp_out on the idle Pool engine
                            # (f32r out; the fin inject applies the x32 via
                            # the scaled identity)
                            nc.gpsimd.tensor_add(
                                y2_sb[:, tt, dc * 512:(dc + 1) * 512],
                                y_sb[:, tt, dc * 512:(dc + 1) * 512],
                                bb[:, dc * 512:(dc + 1) * 512])
                        for i_ in range(4):
                            nc.tensor.matmul(pss_op[(tt, 0)][:], identr[:],
                                             warm_rhs[:],
                                             start=True, stop=True)
                        stats = p3s.tile([128, 2, 6], F32, tag="stats")
                        for g in range(2):
                            nc.vector.bn_stats(
                                stats[:, g, :],
                                y_sb[:, tt, g * 512:(g + 1) * 512])
                        mv = p3s.tile([128, 2], F32, tag="mv")
                        nc.vector.bn_aggr(mv[:], stats[:])
                        sd = p3s.tile([128, 1], F32, tag="sd")
                        nc.scalar.activation(sd[:], mv[:, 1:2], AF.Sqrt,
                                             bias=eps[:], scale=1.0)
                        rstd = p3s.tile([128, 1], F32, tag="rstd")
                        nc.vector.reciprocal(rstd[:], sd[:])
                        nb2 = p3s.tile([128, 1], F32, tag="nb2")
                        nc.vector.scalar_tensor_tensor(
                            nb2[:], mv[:, 0:1], -1.0, rstd[:],
                            MUL, MUL)
                        nc.scalar.activation(hns[:, tt, :], y_sb[:, tt, :],
                                             AF.Identity,
                                             bias=nb2[:], scale=rstd[:])

                # transpose h_norm -> hT8 + hTlo (hi/lo fp8 split: h ~
                # hT8 + hTlo to bf16-class accuracy), then the MLP
                ln_mlp = tc.tile_pool(name="p3acc", bufs=3, space="PSUM")
                p3acc = ln_mlp.__enter__()
                p3pt_ctx = tc.tile_pool(name="p3pt", bufs=2, space="PSUM")
                p3pt = p3pt_ctx.__enter__()
                # prefetch the first mlp-in weight tiles during LN
                wis = {}
                for ff in range(8):
                    wis[ff] = mlpw.tile([128, 2, 8, 128], E4, tag="wi",
                                        name=f"wi{ff}")
                    nc.sync.dma_start(wis[ff][:], w8i_io[:, ff, :, :, :])
                # dc-major: the first mlp-in matmul contracts dc blocks 0-1
                # over ALL tokens, so this order lets the MLP start as soon
                # as the first two dc columns are transposed
                wtm = p3acc.tile([128, 512], F32, tag="acc", name="warmmlp")
                for dc in range(8):
                    for i_ in range(3):   # keep the PE p-state up
                        nc.tensor.matmul(wtm[:], identr[:], warm_rhs[:],
                                         start=True, stop=True)
                    for tt in range(4):
                        pt = p3pt.tile([128, 128], BF, tag="pt3")
                        nc.tensor.transpose(
                            pt[:], hns[:, tt, dc * 128:(dc + 1) * 128],
                            identb[:])
                        tsl = slice(tt * 128, (tt + 1) * 128)
                        # split the psum->fp8 copies across Act and DVE
                        if tt % 2 == 0:
                            nc.scalar.copy(hT8[:, dc, tsl], pt[:])
                        else:
                            nc.vector.tensor_copy(hT8[:, dc, tsl], pt[:])

                # transpose pool freed -> 6-buf mlp-out accumulators so
                # the dc=1 group starts while dc=0 drains
                p3pt_ctx.__exit__(None, None, None)
                p3mo_ctx = tc.tile_pool(name="p3mo", bufs=5, space="PSUM")
                p3mo = p3mo_ctx.__enter__()

                # MLP in + gelu, 3-pass hi/lo DR
                hmT8 = p3sb.tile([128, 32, 512], E4, tag="hmT8")
                hmTlo = p3sb.tile([128, 32, 512], E4, tag="hmTlo")
                for ff in range(32):
                    if ff in wis:
                        wi = wis[ff]
                    else:
                        wi = mlpw.tile([128, 2, 8, 128], E4, tag="wi",
                                       name=f"wi{ff}")
                        nc.sync.dma_start(wi[:], w8i_io[:, ff, :, :, :])
                    ps = p3acc.tile([128, 512], F32, tag="acc")
                    # 2-pass: h8 @ (Wh + Wl) - W at bf16-class accuracy, h
                    # at fp8 (errstudy2 V4: 1.2e-2 vs the 2e-2 gate)
                    first, last = (0, 0), (1, 3)
                    for pi, (wsl, h) in enumerate(
                            ((0, hT8), (1, hT8))):
                        for pp in range(4):
                            nc.tensor.matmul(
                                ps[:], wi[:, wsl, 2 * pp:2 * pp + 2, :],
                                h[:, 2 * pp:2 * pp + 2, :],
                                start=((pi, pp) == first),
                                stop=((pi, pp) == last), perf_mode=DR)
                        if ff == 0:
                            # fill PE idle while transposes/subs complete
                            for i_ in range(3):
                                nc.tensor.matmul(wtm[:], identr[:],
                                                 warm_rhs[:],
                                                 start=True, stop=True)
                    t2 = p3r.tile([128, 512], BF, tag="t2",
                                  name=f"t2_{ff}")
                    nc.scalar.activation(t2[:], ps[:], AF.Gelu,
                                         bias=b_in[:, ff:ff + 1],
                                         scale=1.0 / WS)
                    # hi/lo fp8 split of the gelu output for the DR mlp_out
                    nc.vector.tensor_copy(hmT8[:, ff, :], t2[:])
                    if ff % 2 == 0:
                        nc.vector.tensor_sub(hmTlo[:, ff, :], t2[:],
                                             hmT8[:, ff, :])
                    else:
                        nc.gpsimd.tensor_sub(hmTlo[:, ff, :], t2[:],
                                             hmT8[:, ff, :])

                # MLP out + final residual: 3-pass hi/lo DR over ff-pairs
                out_r = out_io.rearrange("(t p) d -> p t d", p=128)
                for dc in range(2):
                    pss = [p3mo.tile([128, 512], F32, tag="mo",
                                     name=f"mo{dc}_{i}") for i in range(4)]
                    for ffp in range(16):
                        w2 = mlpw.tile([128, 4, 512], E4, tag="w2",
                                       name=f"w2_{dc}_{ffp}")
                        nc.sync.dma_start(
                            w2[:],
                            bass.AP(tensor=w8o_t,
                                    offset=(ffp * 128 * 4 * D + dc * 512),
                                    ap=[[4 * D, 128], [D, 4], [1, 512]]))
                        for tt in range(4):
                            tsl = slice(tt * 128, (tt + 1) * 128)
                            for pi, (g, wsl) in enumerate(
                                    ((hmT8, 0), (hmT8, 1), (hmTlo, 0))):
                                nc.tensor.matmul(
                                    pss[tt][:],
                                    g[:, 2 * ffp:2 * ffp + 2, tsl],
                                    w2[:, 2 * wsl:2 * wsl + 2, :],
                                    start=(ffp == 0 and pi == 0),
                                    stop=False,
                                    perf_mode=DR)
                    for tt in range(4):
                        # inject y2 = WS*(y + b_out); psum carries x32 from
                        # the weight prescale, so one Act scaled copy
                        # finishes the kernel output
                        nc.tensor.matmul(
                            pss[tt][:], identr32[:],
                            y2_sb[:, tt, dc * 512:(dc + 1) * 512],
                            start=False, stop=True)
                        fin = p3s.tile([128, 512], F32, tag="fin")
                        nc.scalar.activation(fin[:], pss[tt][:], AF.Copy,
                                             scale=1.0 / WS)
                        nc.sync.dma_start(
                            out_r[:, tt, dc * 512:(dc + 1) * 512], fin[:])
                for ctx in (p3mo_ctx, ln_mlp):
                    ctx.__exit__(None, None, None)

    nc.compile()
    return nc


def _host_prep(x, alibi, ln1_w, w_qkv, w_out, ln2_w, w_mlp_in, b_mlp_in,
               w_mlp_out, b_mlp_out):
    f32 = np.float32
    x = np.asarray(x, f32)
    x_flat = np.ascontiguousarray(x.reshape(NTOK, D))
    w_qkv = np.asarray(w_qkv, f32)
    w_out = np.asarray(w_out, f32)
    w_mlp_in = np.asarray(w_mlp_in, f32)
    w_mlp_out = np.asarray(w_mlp_out, f32)
    b_mlp_in = np.asarray(b_mlp_in, f32)
    b_mlp_out = np.asarray(b_mlp_out, f32)
    ln2_w = np.asarray(ln2_w, f32)
    alibi = np.asarray(alibi, f32)

    # x8 [p, pp, s, tok] = x[tok, (pp*2+s)*128 + p]
    x8 = np.ascontiguousarray(
        x_flat.T.reshape(4, 2, 128, NTOK).transpose(2, 0, 1, 3)).astype(E4M3)

    # identities
    i240 = (IDENT_V * np.eye(128, dtype=f32)).astype(E4M3)
    identb = np.eye(128, dtype=f32).astype(BF16)

    # out-proj weights: wo8[p, pp, s, dcol] = WS * w_out[dcol, (pp*2+s)*128+p]
    wo8 = np.ascontiguousarray(
        (WS * w_out).T.reshape(4, 2, 128, D).transpose(2, 0, 1, 3)
    ).astype(E4M3)

    w_in_eff = WS * w_mlp_in * ln2_w[None, :]     # (FF, D), x32 for fp8
    w_inP = np.ascontiguousarray(
        w_in_eff.reshape(32, 128, 8, 128).transpose(3, 0, 2, 1))
    w8i_h = w_inP.astype(E4M3)
    w8i_l = (w_inP - w8i_h.astype(np.float32)).astype(E4M3)
    # packed hi/lo: [p, ff, hl, kk, fin]
    w8i = np.ascontiguousarray(
        np.stack([w8i_h, w8i_l], axis=2))
    # w2P [ffp, p, s, dcol] = (32*w_mlp_out)[dcol, (ffp*2+s)*128+p]
    w2P = np.ascontiguousarray(
        (WS * w_mlp_out).T.reshape(16, 2, 128, D).transpose(0, 2, 1, 3))
    w8o_h = w2P.astype(E4M3)
    w8o_l = (w2P - w8o_h.astype(np.float32)).astype(E4M3)
    # packed hi/lo: [ffp, p, hl, s, dcol]
    w8o = np.ascontiguousarray(np.stack([w8o_h, w8o_l], axis=2))
    b_inT = np.ascontiguousarray(b_mlp_in.reshape(32, 128).T)

    in_maps = []
    for c in range(NCORES):
        h0 = HPC * c
        qrows = WS * w_qkv[h0 * Dh:(h0 + HPC) * Dh]
        krows = WS * w_qkv[H * Dh + h0 * Dh:H * Dh + (h0 + HPC) * Dh]
        vrows = WS * w_qkv[2 * H * Dh + h0 * Dh:2 * H * Dh + (h0 + HPC) * Dh]
        wq = np.concatenate([qrows, krows, vrows], 0)   # [384, D]
        # wq8 [p, pp, s, mcol] = wq[mcol, (pp*2+s)*128 + p]
        wq8 = np.ascontiguousarray(
            wq.T.reshape(4, 2, 128, 384).transpose(2, 0, 1, 3)).astype(E4M3)
        # alibi transposed to [hl, k, q]; k-row blocks gathered per engine
        # assignment (al8 row-block i = ACT_KTS[i], ab16 row-block i =
        # DVE_KTS[i])
        alc = alibi[0, h0:h0 + HPC].transpose(0, 2, 1)  # [HPC, T(k), T(q)]
        alc_kt = alc.reshape(HPC, KT, 128, T)
        al8 = np.ascontiguousarray(
            AL_SCALE * alc_kt[:, ACT_KTS].reshape(HPC, NAK * 128, T)
        ).astype(E4M3)
        ab16 = np.ascontiguousarray(
            LOG2E8 * alc_kt[:, DVE_KTS].reshape(HPC, NBK * 128, T) + BT_BIAS
        ).astype(np.float16)
        # residual pre-scaled x32: it is injected into the x32-scaled
        # out-proj / mlp-out PSUMs via f32r identity matmuls
        x_res = np.ascontiguousarray(WS * x_flat[c * CHUNK:(c + 1) * CHUNK])
        in_maps.append({
            "x8": x8, "wq8": wq8, "al8": al8, "ab16": ab16, "i240": i240,
            "identb": identb, "wo8": wo8,
            "x_res": x_res, "b_outT": b_mlp_out.reshape(1, D),
            "w8i": w8i, "b_inT": b_inT, "w8o": w8o,
        })
    return in_maps


def _get_compiled():
    global _COMPILED
    if _COMPILED is None:
        _COMPILED = _build()
    return _COMPILED


def kernel(_trace=False, **inputs):
    nc = _get_compiled()
    in_maps = _host_prep(**inputs)
    res = None
    for attempt in range(3):
        try:
            res = run_bass_kernel_spmd(nc, in_maps,
                                       core_ids=list(range(NCORES)),
                                       trace=_trace)
            break
        except Exception:
            if attempt == 2:
                raise
    out = np.concatenate([res.results[c]["out"] for c in range(NCORES)], 0)
    out = out.reshape(B, T, D).astype(np.float32)
    if _trace:
        return out, res
    return out


# revision 78
# speedup vs baseline: 1.2686x; 1.0061x over previous
"""Trainium2 Bass kernel for nn_AttentionBlock (B=2, T=2048, D=1024, H=16,
Dh=64, Ff=4096), SPMD across 8 NeuronCores in one NEFF launch.

Sharding:
  - Phase 1+2 (QKV projection + attention): 2 heads per core over all 4096
    tokens. Phase 3 (out-proj + residual + LayerNorm + MLP): 512 tokens per
    core, re-sharded via an AllToAll of the attention output (fp8, 0.5 MiB).

Numerics (validated end-to-end in numpy against the reference, see
errstudy2.py: ~2e-3 max-rel vs the 2e-2 gate):
  - Attention matmuls run fp8e4m3 DoubleRow as before (QKV projection,
    Q.K^T scores, attn@V, output projection). Weights pre-scaled x32.
  - The softmax exp is split across two engines per k-tile (the Act engine
    was the attention bottleneck at ~1us per [128,1024] exp):
      * kt < NAK: alibi injected into the scores PSUM by a plain fp8 matmul
        against a 240-valued identity (as before), then Act exp ->
        fp8 attention weights.
      * kt >= NAK: DVE computes the fp8 weights directly via the
        exp bit-trick: i8 = psum*(8*log2e/8192) + (alibi*8*log2e + 56 [+rnd])
        written as int8 and bit-viewed as e4m3 ~= exp(s+alibi). The f16
        alibi-bias tile fuses the alibi add, so no PE inject is needed.
    The bit-trick's mantissa-interpolation error (~3%) is the same class as
    the e4m3 quantization the Act path already incurs; softmax averaging
    washes both out (errstudy2: 2.01e-3 vs 2.13e-3).
  - Scores DoubleRow uses a zeroed second k-tile slot; attn@V pairs adjacent
    score k-tiles. V padded to 128 cols (col 64 = softmax-denominator ones
    column = 32).
  - mlp_in runs 2-pass (h8 @ (W_hi + W_lo): weights at bf16-class accuracy,
    activations fp8); mlp_out stays 3-pass hi/lo. End-to-end rel_absmax
    measured 1.204e-2 on HW (errstudy2 V4 predicted 1.206e-2) vs the 2e-2
    gate. The A2A is split by head-half so the first collective overlaps
    the second half of attention, and the out-projection runs in two
    partition-half stages with the residual injected via f32r identity
    matmuls (readouts become pure Act/DVE scaled copies).

kernel(**inputs) takes FULL unsharded inputs, returns the FULL output.
"""

import sys

for _p in ("/opt/trn_rl_repo", "/root/.axon_site/_ro/trn_rl_repo"):
    if _p not in sys.path:
        sys.path.insert(0, _p)

import numpy as np
import ml_dtypes

import concourse.bass as bass
import concourse.tile as tile
from concourse import bacc, mybir
from concourse.bass_utils import run_bass_kernel_spmd

BF16 = ml_dtypes.bfloat16
E4M3 = ml_dtypes.float8_e4m3   # TRN float8e4: max 240

B, T, D, H, Dh, FF = 2, 2048, 1024, 16, 64, 4096
NTOK = B * T            # 4096
NCORES = 8
CHUNK = NTOK // NCORES  # 512 tokens per core
HPC = H // NCORES       # 2 heads per core
KT = T // 128           # 16 k-tiles per batch

# Interleaved engine assignment of the 16 k-tiles per pass: Act tiles get
# true exp (+PE alibi inject), DVE tiles get the fused bit-trick exp.
# Interleaving keeps both engines fed as the PE emits scores in kt order.
ACT_KTS = (0, 2, 4, 6, 8, 10, 12, 14)
DVE_KTS = (1, 3, 5, 7, 9, 11, 13, 15)
NAK = len(ACT_KTS)
NBK = len(DVE_KTS)
A_POS = {kt: i for i, kt in enumerate(ACT_KTS)}
D_POS = {kt: i for i, kt in enumerate(DVE_KTS)}

WS = 32.0               # host weight pre-scale for fp8 range
EXP_SCALE = 1.0 / (WS * WS * 8.0)   # 1/8192: undo q,k x32 and apply 1/sqrt(Dh)
IDENT_V = 240.0                      # e4m3-exact identity magnitude
AL_SCALE = (WS * WS * 8.0) / IDENT_V  # alibi pre-scale so PSUM alibi = 8192*a
LOG2E8 = float(8.0 * np.log2(np.e))  # e4m3 bit-trick exp slope
BT_SCALE = LOG2E8 / 8192.0
BT_BIAS = 56.0          # e4m3 exponent-bias term (see _host_prep; rounding
                        # offset calibrated on HW via probe.py)

F32 = mybir.dt.float32
F32R = mybir.dt.float32r
BF = mybir.dt.bfloat16
E4 = mybir.dt.float8e4
F16 = mybir.dt.float16
I8 = mybir.dt.int8
AF = mybir.ActivationFunctionType
DR = mybir.MatmulPerfMode.DoubleRow
MUL = mybir.AluOpType.mult
ADD = mybir.AluOpType.add

_COMPILED = None


def _build(sim1=False):
    nc = bacc.Bacc("TRN2", target_bir_lowering=False, debug=False,
                   num_devices=1 if sim1 else NCORES)

    # ---- kernel I/O (per core) ----
    # x packed [p, pp, s, tok] = x[tok, (pp*2+s)*128 + p], fp8
    x8_io = nc.dram_tensor("x8", [128, 4, 2, NTOK], E4, kind="ExternalInput").ap()
    # qkv weights x32 packed [p, pp, s, m*128+j] = w[row(m,j), (pp*2+s)*128+p]
    wq8_io = nc.dram_tensor("wq8", [128, 4, 2, 384], E4, kind="ExternalInput").ap()
    # alibi*AL_SCALE transposed, Act k-tiles only: al8[hl, k, q], k < NAK*128
    al8_t = nc.dram_tensor("al8", [HPC, NAK * 128, T], E4, kind="ExternalInput")
    # bit-trick alibi bias, f16: ab16[hl, k - NAK*128, q]
    ab16_t = nc.dram_tensor("ab16", [HPC, NBK * 128, T], F16,
                            kind="ExternalInput")
    # identity of value IDENT_V for the alibi inject matmul
    i240_io = nc.dram_tensor("i240", [128, 128], E4, kind="ExternalInput").ap()
    identb_io = nc.dram_tensor("identb", [128, 128], BF, kind="ExternalInput").ap()
    # out-proj weights x32 packed [p, pp, s, dcol] = WS*w_out[dcol, (pp*2+s)*128+p]
    wo8_io = nc.dram_tensor("wo8", [128, 4, 2, D], E4, kind="ExternalInput").ap()
    x_res_io = nc.dram_tensor("x_res", [CHUNK, D], F32, kind="ExternalInput").ap()
    b_outT_io = nc.dram_tensor("b_outT", [1, D], F32, kind="ExternalInput").ap()
    # mlp-in weights, hi/lo packed in one tensor:
    # [p, ff, hl, kk, fin] = hi/lo of (32*w_in_eff)[ff*128+fin, kk*128+p]
    w8i_io = nc.dram_tensor("w8i", [128, 32, 2, 8, 128], E4,
                            kind="ExternalInput").ap()
    b_inT_io = nc.dram_tensor("b_inT", [128, 32], F32, kind="ExternalInput").ap()
    # mlp-out weights, hi/lo packed: [ffp, p, hl, s, dcol] =
    # hi/lo of (32*w_mlp_out)[dcol, (ffp*2+s)*128+p]
    w8o_t = nc.dram_tensor("w8o", [16, 128, 2, 2, D], E4,
                           kind="ExternalInput")
    out_io = nc.dram_tensor("out", [CHUNK, D], F32, kind="ExternalOutput").ap()

    # ---- internal DRAM ----
    # per-head-half collective buffers: A2A#0 fires after the hl=0 passes
    # and overlaps the hl=1 attention; A2A#1 covers the rest.
    cc_send = [nc.dram_tensor(f"cc_send{hl}", [D // 2, CHUNK], E4)
               for hl in range(2)]
    cc_recv = [nc.dram_tensor(f"cc_recv{hl}", [D // 2, CHUNK], E4)
               for hl in range(2)]

    with tile.TileContext(nc) as tc:
        with tc.tile_pool(name="consts", bufs=1) as consts:
            i240 = consts.tile([128, 128], E4, tag="i240")
            nc.sync.dma_start(i240[:], i240_io)
            identb = consts.tile([128, 128], BF, tag="identb")
            nc.sync.dma_start(identb[:], identb_io)
            warm_f = consts.tile([128, 512], F32, tag="warm_f")
            nc.vector.memset(warm_f[:], 0.5)
            warm_rhs = consts.tile([128, 512], F32R, tag="warm_rhs")
            nc.vector.tensor_copy(warm_rhs[:], warm_f[:])
            identr = consts.tile([128, 128], F32R, tag="identr")
            nc.vector.tensor_copy(identr[:], identb[:])
            # 32-scaled f32r identity: injects the (unscaled) y2 residual
            # into the x32-scaled mlp-out PSUM
            identr32 = consts.tile([128, 128], F32R, tag="identr32")
            nc.vector.tensor_scalar(identr32[:], identb[:], WS, None, MUL)
            # phase-3 input tiles (loaded mid-attention; consts pool lives
            # for the whole kernel)
            wo8 = consts.tile([128, 4, 2, D], E4, tag="wo8")
            b_in = consts.tile([128, 32], F32, tag="b_in")
            bob = consts.tile([1, D], F32, tag="bob")
            xrs = consts.tile([128, 4, D], F32, tag="xr")
            # f32r copy for the residual-inject matmul (fp32r operands must
            # be produced rounded; idle-Pool copies during attention)
            xrsr = consts.tile([128, 4, D], F32R, tag="xrr")
            # re-sharded attention output, assembled per head-half as each
            # AllToAll lands (partitions 0-63 <- hl=0, 64-127 <- hl=1)
            yrT = consts.tile([128, 8, 512], E4, tag="yrT")

            with tc.tile_pool(name="qkv", bufs=1) as qkv, \
                 tc.tile_pool(name="alb", bufs=3) as albp, \
                 tc.tile_pool(name="abb", bufs=3) as abbp:
                al_cache = {}

                def load_alibi(hl, qc, ring, splits=1):
                    # ring choice controls WHEN the transfer runs relative
                    # to the x8 loads on the shared DMA engines. splits>1
                    # halves the first pass's wait: its early k-tiles'
                    # alibi lands while the rest still streams.
                    alt = albp.tile([128, NAK, 1024], E4, tag="al",
                                    name=f"al{hl}_{qc}")
                    abt = abbp.tile([128, NBK, 1024], F16, tag="ab",
                                    name=f"ab{hl}_{qc}")
                    cuts_a = [NAK * i // splits for i in range(splits + 1)]
                    cuts_b = [NBK * i // splits for i in range(splits + 1)]
                    for s in range(splits):
                        a0, a1 = cuts_a[s], cuts_a[s + 1]
                        ring.dma_start(
                            alt[:, a0:a1, :],
                            bass.AP(tensor=al8_t,
                                    offset=(hl * NAK + a0) * 128 * T
                                    + qc * 1024,
                                    ap=[[T, 128], [128 * T, a1 - a0],
                                        [1, 1024]]))
                        b0_, b1_ = cuts_b[s], cuts_b[s + 1]
                        ring.dma_start(
                            abt[:, b0_:b1_, :],
                            bass.AP(tensor=ab16_t,
                                    offset=(hl * NBK + b0_) * 128 * T
                                    + qc * 1024,
                                    ap=[[T, 128], [128 * T, b1_ - b0_],
                                        [1, 1024]]))
                    al_cache[(hl, qc)] = (alt, abt)

                q8s, k8s, v8s = [], [], []
                for b in range(2):
                    q8 = qkv.tile([128, 2, T], E4, tag=f"q8{b}", name=f"q8{b}")
                    k8 = qkv.tile([128, 2, T], E4, tag=f"k8{b}", name=f"k8{b}")
                    # v8 [p, ktpair, slot, hl, 128]: cols 0-63 v, col 64 = WS
                    # (softmax denominator), cols 65-127 zero (DR stationary
                    # width must be 32/64/128); zeroed first - the v8 copies
                    # need them before the q8/k8 zero-slots are read
                    v8 = qkv.tile([128, 8, 2, 2, 128], E4, tag=f"v8{b}",
                                  name=f"v8{b}")
                    nc.gpsimd.memset(v8[:, :, :, :, 64:], 0.0)
                    nc.vector.memset(v8[:, :, :, :, 64:65], WS)
                    q8s.append(q8); k8s.append(k8); v8s.append(v8)
                for b in range(2):
                    # zero-slot for scores DR second k-tile
                    nc.gpsimd.memset(k8s[b][:, 1, :], 0.0)
                    nc.gpsimd.memset(q8s[b][:, 1, :], 0.0)

                # ---------------- phase 1: QKV projection ----------------
                with tc.tile_pool(name="p1x", bufs=2) as p1x, \
                     tc.tile_pool(name="p1w", bufs=1) as p1w, \
                     tc.tile_pool(name="p1t", bufs=3) as p1t, \
                     tc.tile_pool(name="p1ps", bufs=6, space="PSUM") as p1ps, \
                     tc.tile_pool(name="p1pt", bufs=2, space="PSUM") as p1pt:
                    wq8 = p1w.tile([128, 4, 2, 384], E4, tag="wq8")
                    nc.sync.dma_start(wq8[:], wq8_io)
                    for wi_ in range(4):
                        wps = p1pt.tile([128, 512], F32, tag="pt",
                                        name=f"warms{wi_}")
                        nc.tensor.matmul(wps[:], identr[:], warm_rhs[:],
                                         start=True, stop=True)

                    # x loads first (proj critical path), alibi loads woven
                    # in between so they land before their passes without
                    # delaying the projection on the shared DMA engines
                    xtss = []
                    for b in range(2):
                        xts = p1x.tile([128, 4, 2, 2048], E4, tag="xt",
                                       name=f"xt{b}")
                        xtss.append(xts)
                        for cc4 in range(4):
                            nc.sync.dma_start(
                                xts[:, :, :, cc4 * 512:(cc4 + 1) * 512],
                                x8_io[:, :, :,
                                      b * 2048 + cc4 * 512:
                                      b * 2048 + (cc4 + 1) * 512])
                    load_alibi(0, 0, nc.sync, splits=3)
                    load_alibi(0, 1, nc.sync)

                    for b in range(2):
                        q8, k8, v8 = q8s[b], k8s[b], v8s[b]
                        xts = xtss[b]
                        with nc.named_scope(f"qkvproj{b}"):
                            for t in range(4):
                                tsl = slice(t * 512, (t + 1) * 512)
                                for m in range(3):   # q, k, v
                                    ps = p1ps.tile([128, 512], F32, tag="proj",
                                                   name=f"proj{b}_{t}_{m}")
                                    for pp in range(4):
                                        nc.tensor.matmul(
                                            ps[:],
                                            wq8[:, pp, :, m * 128:(m + 1) * 128],
                                            xts[:, pp, :, tsl],
                                            start=(pp == 0), stop=(pp == 3),
                                            perf_mode=DR)
                                    if m == 0:
                                        nc.scalar.copy(q8[:, 0, tsl], ps[:])
                                    elif m == 1:
                                        nc.vector.tensor_copy(k8[:, 0, tsl],
                                                              ps[:])
                                    else:
                                        vt = p1t.tile([128, 512], BF, tag="vt",
                                                      name=f"vt{b}_{t}")
                                        nc.scalar.copy(vt[:], ps[:])
                                        for j in range(4):
                                            ti = t * 4 + j
                                            pt = p1pt.tile([128, 128], BF,
                                                           tag="pt",
                                                           name=f"pt{b}_{ti}")
                                            nc.tensor.transpose(
                                                pt[:],
                                                vt[:, j * 128:(j + 1) * 128],
                                                identb[:])
                                            nc.vector.tensor_copy(
                                                v8[:, ti // 2, ti % 2, :, 0:64],
                                                pt[:].rearrange(
                                                    "p (h d) -> p h d", h=2))
                        if b == 1:
                            # sync ring: the pool rotation (bufs=3) holds
                            # the 4th tile until pass (0,0,1) completes, so
                            # these transfers naturally run mid-attention
                            # without competing with the x8/al(0,*) loads;
                            # nothing urgent sits behind them on SP
                            load_alibi(1, 0, nc.sync)
                            load_alibi(1, 1, nc.sync)

                # ---------------- phase 2: attention ----------------
                with nc.named_scope("attn"), \
                     tc.tile_pool(name="exps", bufs=8) as expp, \
                     tc.tile_pool(name="sps", bufs=3, space="PSUM") as spsp, \
                     tc.tile_pool(name="yups", bufs=1, space="PSUM") as yupp, \
                     tc.tile_pool(name="nrm", bufs=2) as nrmp, \
                     tc.tile_pool(name="yns", bufs=4) as ynp:
                    # each pass's final yn multiply is emitted mid-NEXT-pass
                    # so the in-order DVE queue never stalls on the Pool
                    # broadcast it depends on
                    pending_norm = []

                    def flush_norm():
                        while pending_norm:
                            pending_norm.pop(0)()

                    def attn_pass(hl, qc, b):
                        alt, abt = al_cache[(hl, qc)]
                        hsl = slice(hl * 64, (hl + 1) * 64)
                        yu = yupp.tile([128, 1024], F32, tag="yu",
                                       name=f"yu{hl}_{qc}_{b}")
                        ex8 = None
                        exq = []  # deferred attnV ops: (ktp, ex8)
                        for kt in range(KT):
                            is_act = kt in A_POS
                            sp = spsp.tile([128, 1024], F32, tag="sp",
                                           name=f"sp{hl}_{qc}_{b}_{kt}")
                            for qh in range(2):
                                qsl = slice(qc * 1024 + qh * 512,
                                            qc * 1024 + (qh + 1) * 512)
                                osl = slice(qh * 512, (qh + 1) * 512)
                                nc.tensor.matmul(
                                    sp[:, osl],
                                    k8s[b][hsl, :, kt * 128:(kt + 1) * 128],
                                    q8s[b][hsl, :, qsl],
                                    start=True, stop=not is_act,
                                    perf_mode=DR)
                                if is_act:
                                    nc.tensor.matmul(
                                        sp[:, osl],
                                        i240[:],
                                        alt[:, A_POS[kt], osl],
                                        start=False, stop=True)
                            if kt % 2 == 0:
                                ex8 = expp.tile([128, 2, 1024], E4, tag="ex",
                                                name=f"ex{hl}_{qc}_{b}_{kt}")
                            if is_act:
                                nc.scalar.activation(ex8[:, kt % 2, :], sp[:],
                                                     AF.Exp, scale=EXP_SCALE)
                            else:
                                nc.vector.scalar_tensor_tensor(
                                    ex8[:, kt % 2, :].bitcast(I8), sp[:],
                                    BT_SCALE, abt[:, D_POS[kt], :], MUL, ADD)
                            if kt == 5:
                                # previous pass's deferred yn multiply: by
                                # now its Pool broadcast has finished, so
                                # the DVE never blocks on it
                                flush_norm()
                            if kt % 2 == 1:
                                exq.append((kt // 2, ex8))
                            # drain attnV four k-tile-pairs behind the scores
                            # so the PE queue never waits on a fresh exp nor
                            # on the previous pass's norm freeing yu
                            while exq and (exq[0][0] <= kt // 2 - 4
                                           or kt == KT - 1):
                                ktp, exd = exq.pop(0)
                                for qh in range(2):
                                    osl = slice(qh * 512, (qh + 1) * 512)
                                    nc.tensor.matmul(
                                        yu[:, osl],
                                        v8s[b][:, ktp, :, hl, :],
                                        exd[:, :, osl],
                                        start=(ktp == 0), stop=(ktp == 7),
                                        perf_mode=DR)
                        rec = nrmp.tile([1, 1024], F32, tag="rec",
                                        name=f"rec{hl}_{qc}_{b}")
                        nc.vector.reciprocal(rec[:], yu[64:65, :])
                        bc = nrmp.tile([64, 1024], F32, tag="bc",
                                       name=f"bc{hl}_{qc}_{b}")
                        nc.gpsimd.partition_broadcast(bc[:], rec[:])

                        def norm_fin(hl=hl, qc=qc, b=b, yu=yu, bc=bc):
                            i = b * 2 + qc
                            yn = ynp.tile([64, 1024], E4, tag="yn",
                                          name=f"yn{hl}_{qc}_{b}")
                            nc.vector.tensor_mul(yn[:], yu[0:64, :], bc[:])
                            nc.sync.dma_start(
                                bass.AP(tensor=cc_send[hl],
                                        offset=2 * i * 64 * 512,
                                        ap=[[512, 64], [64 * 512, 2],
                                            [1, 512]]),
                                yn[:].rearrange("p (h c) -> p h c", h=2))
                        pending_norm.append(norm_fin)

                    def run_a2a(hl):
                        with nc.named_scope(f"a2a{hl}"):
                            if sim1:
                                nc.sync.dma_start(cc_recv[hl][:],
                                                  cc_send[hl][:])
                            else:
                                nc.gpsimd.collective_compute(
                                    "AllToAll", mybir.AluOpType.bypass,
                                    replica_groups=[list(range(NCORES))],
                                    ins=[cc_send[hl][:]],
                                    outs=[cc_recv[hl][:]])
                            # assemble the hl half of yrT: partitions
                            # hl*64.. <- recv blocks [64, 512] per src core
                            # (sync ring: a scalar-ring DGE would block the
                            # Act queue's exps until the collective lands)
                            nc.sync.dma_start(
                                yrT[hl * 64:(hl + 1) * 64, :, :],
                                bass.AP(tensor=cc_recv[hl], offset=0,
                                        ap=[[512, 64], [64 * 512, 8],
                                            [1, 512]]))

                    def pe_keepwarm(n, pool, tag, name):
                        # bridge PE idle gaps so the p-state never drops
                        wt = pool.tile([128, 1024], F32, tag=tag, name=name)
                        for i_ in range(n):
                            nc.tensor.matmul(wt[:, 0:512], identr[:],
                                             warm_rhs[:],
                                             start=True, stop=True)

                    first = True
                    for hl in range(2):
                        for qc in range(2):
                            for b in range(2):
                                attn_pass(hl, qc, b)
                                if first:
                                    # no-dependency phase-3 loads: issued here
                                    # so they transfer during attention
                                    nc.sync.dma_start(wo8[:], wo8_io[:])
                                    nc.sync.dma_start(b_in[:], b_inT_io[:])
                                    nc.sync.dma_start(bob[:], b_outT_io[:])
                                    x_res_r = x_res_io.rearrange(
                                        "(t p) d -> p t d", p=128)
                                    for tt_ in range(4):
                                        nc.sync.dma_start(xrs[:, tt_, :],
                                                          x_res_r[:, tt_, :])
                                        nc.gpsimd.tensor_copy(
                                            xrsr[:, tt_, :], xrs[:, tt_, :])
                                    first = False
                        if hl == 0:
                            # A2A#0 emission rides the deferred-norm queue:
                            # it lands right after pass 3's yn DMA, inside
                            # pass 4's DVE stream (no hl-boundary stall)
                            pending_norm.append(lambda: run_a2a(0))
                        else:
                            pe_keepwarm(13, spsp, "sp", "warm_a2a")
                            flush_norm()
                            run_a2a(1)

            # ---------------- phase 3: out-proj + LN + MLP ----------------
            with nc.named_scope("mlp"), \
                 tc.tile_pool(name="p3sb", bufs=1) as p3sb, \
                 tc.tile_pool(name="p3r", bufs=3) as p3r, \
                 tc.tile_pool(name="p3s", bufs=4) as p3s, \
                 tc.tile_pool(name="mlpw", bufs=8) as mlpw:
                # dummy Sqrt during the A2A hole: pulls the Act
                # function-table swap off the preamble critical path
                dum = p3s.tile([128, 1], F32, tag="dum")
                nc.scalar.activation(dum[:], warm_f[:, 0:1], AF.Sqrt,
                                     scale=1.0)
                bb = p3sb.tile([128, D], F32, tag="bb")
                nc.gpsimd.partition_broadcast(bb[:], bob[:])

                y_sb = p3sb.tile([128, 4, D], F32, tag="y_sb")
                # f32r: written rounded by the Pool stt, consumed by the
                # final-residual inject matmul
                y2_sb = p3sb.tile([128, 4, D], F32R, tag="y2_sb")
                hns = p3sb.tile([128, 4, D], BF, tag="hns")
                hT8 = p3sb.tile([128, 8, 512], E4, tag="hT8")
                eps = p3sb.tile([128, 1], F32, tag="eps")
                nc.vector.memset(eps[:], 1e-5)
                # 2-stage out-proj: the hl=0 partials (partitions 0-63 of
                # each pp slot) run as soon as A2A#0 lands, overlapping the
                # A2A#1 transfer; stage 2 accumulates the hl=1 rows and
                # injects the x32-scaled residual via an f32r identity
                # matmul, so the readout is a pure scaled copy.
                with tc.tile_pool(name="p3acc8", bufs=8,
                                  space="PSUM") as p3acc8:
                    pss_op = {}
                    for hstage in range(2):
                        psl = slice(hstage * 64, (hstage + 1) * 64)
                        for tt in range(4):
                            for dc in range(2):
                                if hstage == 0:
                                    pss_op[(tt, dc)] = p3acc8.tile(
                                        [128, 512], F32, tag="acc",
                                        name=f"acc{tt}_{dc}")
                                ps = pss_op[(tt, dc)]
                                for pp in range(4):
                                    nc.tensor.matmul(
                                        ps[:],
                                        yrT[psl, 2 * pp:2 * pp + 2,
                                            tt * 128:(tt + 1) * 128],
                                        wo8[psl, pp, :,
                                            dc * 512:(dc + 1) * 512],
                                        start=(hstage == 0 and pp == 0),
                                        stop=False,
                                        perf_mode=DR)
                                if hstage == 1:
                                    nc.tensor.matmul(
                                        ps[:], identr[:],
                                        xrsr[:, tt, dc * 512:(dc + 1) * 512],
                                        start=False, stop=True)
                    # readouts interleaved with the LN chains (dc0 on Act,
                    # dc1 on DVE) so the in-order engine queues dovetail;
                    # warm matmuls into just-read accumulators keep the PE
                    # p-state up through this window
                    for tt in range(4):
                        nc.scalar.activation(
                            y_sb[:, tt, 0:512], pss_op[(tt, 0)][:],
                            AF.Copy, scale=1.0 / WS)
                        nc.vector.tensor_scalar(
                            y_sb[:, tt, 512:1024], pss_op[(tt, 1)][:],
                            1.0 / WS, None, MUL)
                        for dc in range(2):
                            # y2 = y + b_mlp_out on the idle Pool engine
                            # (f32r out; the fin inject applies the x32 via
                            # the scaled identity)
                            nc.gpsimd.tensor_add(
                                y2_sb[:, tt, dc * 512:(dc + 1) * 512],
                                y_sb[:, tt, dc * 512:(dc + 1) * 512],
                                bb[:, dc * 512:(dc + 1) * 512])
                        for i_ in range(4):
                            nc.tensor.matmul(pss_op[(tt, 0)][:], identr[:],
                                             warm_rhs[:],
                                             start=True, stop=True)
                        stats = p3s.tile([128, 2, 6], F32, tag="stats")
                        for g in range(2):
                            nc.vector.bn_stats(
                                stats[:, g, :],
                                y_sb[:, tt, g * 512:(g + 1) * 512])
                        mv = p3s.tile([128, 2], F32, tag="mv")
                        nc.vector.bn_aggr(mv[:], stats[:])
                        sd = p3s.tile([128, 1], F32, tag="sd")
                        nc.scalar.activation(sd[:], mv[:, 1:2], AF.Sqrt,
                                             bias=eps[:], scale=1.0)
                        rstd = p3s.tile([128, 1], F32, tag="rstd")
                        nc.vector.reciprocal(rstd[:], sd[:])
                        nb2 = p3s.tile([128, 1], F32, tag="nb2")
                        nc.vector.scalar_tensor_tensor(
                            nb2[:], mv[:, 0:1], -1.0, rstd[:],
                            MUL, MUL)
                        nc.scalar.activation(hns[:, tt, :], y_sb[:, tt, :],
                                             AF.Identity,
                                             bias=nb2[:], scale=rstd[:])

                # transpose h_norm -> hT8 + hTlo (hi/lo fp8 split: h ~
                # hT8 + hTlo to bf16-class accuracy), then the MLP
                ln_mlp = tc.tile_pool(name="p3acc", bufs=3, space="PSUM")
                p3acc = ln_mlp.__enter__()
                p3pt_ctx = tc.tile_pool(name="p3pt", bufs=2, space="PSUM")
                p3pt = p3pt_ctx.__enter__()
                # prefetch the first mlp-in weight tiles during LN
                wis = {}
                for ff in range(8):
                    wis[ff] = mlpw.tile([128, 2, 8, 128], E4, tag="wi",
                                        name=f"wi{ff}")
                    nc.sync.dma_start(wis[ff][:], w8i_io[:, ff, :, :, :])
                # dc-major: the first mlp-in matmul contracts dc blocks 0-1
                # over ALL tokens, so this order lets the MLP start as soon
                # as the first two dc columns are transposed
                wtm = p3acc.tile([128, 512], F32, tag="acc", name="warmmlp")
                for dc in range(8):
                    for i_ in range(3):   # keep the PE p-state up
                        nc.tensor.matmul(wtm[:], identr[:], warm_rhs[:],
                                         start=True, stop=True)
                    for tt in range(4):
                        pt = p3pt.tile([128, 128], BF, tag="pt3")
                        nc.tensor.transpose(
                            pt[:], hns[:, tt, dc * 128:(dc + 1) * 128],
                            identb[:])
                        tsl = slice(tt * 128, (tt + 1) * 128)
                        # split the psum->fp8 copies across Act and DVE
                        if tt % 2 == 0:
                            nc.scalar.copy(hT8[:, dc, tsl], pt[:])
                        else:
                            nc.vector.tensor_copy(hT8[:, dc, tsl], pt[:])

                # transpose pool freed -> 6-buf mlp-out accumulators so
                # the dc=1 group starts while dc=0 drains
                p3pt_ctx.__exit__(None, None, None)
                p3mo_ctx = tc.tile_pool(name="p3mo", bufs=5, space="PSUM")
                p3mo = p3mo_ctx.__enter__()

                # MLP in + gelu, 3-pass hi/lo DR
                hmT8 = p3sb.tile([128, 32, 512], E4, tag="hmT8")
                hmTlo = p3sb.tile([128, 32, 512], E4, tag="hmTlo")
                for ff in range(32):
                    if ff in wis:
                        wi = wis[ff]
                    else:
                        wi = mlpw.tile([128, 2, 8, 128], E4, tag="wi",
                                       name=f"wi{ff}")
                        nc.sync.dma_start(wi[:], w8i_io[:, ff, :, :, :])
                    ps = p3acc.tile([128, 512], F32, tag="acc")
                    # 2-pass: h8 @ (Wh + Wl) - W at bf16-class accuracy, h
                    # at fp8 (errstudy2 V4: 1.2e-2 vs the 2e-2 gate)
                    first, last = (0, 0), (1, 3)
                    for pi, (wsl, h) in enumerate(
                            ((0, hT8), (1, hT8))):
                        for pp in range(4):
                            nc.tensor.matmul(
                                ps[:], wi[:, wsl, 2 * pp:2 * pp + 2, :],
                                h[:, 2 * pp:2 * pp + 2, :],
                                start=((pi, pp) == first),
                                stop=((pi, pp) == last), perf_mode=DR)
                        if ff == 0:
                            # fill PE idle while transposes/subs complete
                            for i_ in range(3):
                                nc.tensor.matmul(wtm[:], identr[:],
                                                 warm_rhs[:],
                                                 start=True, stop=True)
                    t2 = p3r.tile([128, 512], BF, tag="t2",
                                  name=f"t2_{ff}")
                    nc.scalar.activation(t2[:], ps[:], AF.Gelu,
                                         bias=b_in[:, ff:ff + 1],
                                         scale=1.0 / WS)
                    # hi/lo fp8 split of the gelu output for the DR mlp_out
                    nc.vector.tensor_copy(hmT8[:, ff, :], t2[:])
                    if ff % 2 == 0:
                        nc.vector.tensor_sub(hmTlo[:, ff, :], t2[:],
                                             hmT8[:, ff, :])
                    else:
                        nc.gpsimd.tensor_sub(hmTlo[:, ff, :], t2[:],
                                             hmT8[:, ff, :])

                # MLP out + final residual: 3-pass hi/lo DR over ff-pairs
                out_r = out_io.rearrange("(t p) d -> p t d", p=128)
                for dc in range(2):
                    pss = [p3mo.tile([128, 512], F32, tag="mo",
                                     name=f"mo{dc}_{i}") for i in range(4)]
                    for ffp in range(16):
                        w2 = mlpw.tile([128, 4, 512], E4, tag="w2",
                                       name=f"w2_{dc}_{ffp}")
                        nc.sync.dma_start(
                            w2[:],
                            bass.AP(tensor=w8o_t,
                                    offset=(ffp * 128 * 4 * D + dc * 512),
                                    ap=[[4 * D, 128], [D, 4], [1, 512]]))
                        for tt in range(4):
                            tsl = slice(tt * 128, (tt + 1) * 128)
                            for pi, (g, wsl) in enumerate(
                                    ((hmT8, 0), (hmT8, 1), (hmTlo, 0))):
                                nc.tensor.matmul(
                                    pss[tt][:],
                                    g[:, 2 * ffp:2 * ffp + 2, tsl],
                                    w2[:, 2 * wsl:2 * wsl + 2, :],
                                    start=(ffp == 0 and pi == 0),
                                    stop=False,
                                    perf_mode=DR)
                    for tt in range(4):
                        # inject y2 = WS*(y + b_out); psum carries x32 from
                        # the weight prescale, so one Act scaled copy
                        # finishes the kernel output
                        nc.tensor.matmul(
                            pss[tt][:], identr32[:],
                            y2_sb[:, tt, dc * 512:(dc + 1) * 512],
                            start=False, stop=True)
                        fin = p3s.tile([128, 512], F32, tag="fin")
                        nc.scalar.activation(fin[:], pss[tt][:], AF.Copy,
                                             scale=1.0 / WS)
                        nc.sync.dma_start(
                            out_r[:, tt, dc * 512:(dc + 1) * 512], fin[:])
                for ctx in (p3mo_ctx, ln_mlp):
                    ctx.__exit__(None, None, None)

    nc.compile()
    return nc


def _host_prep(x, alibi, ln1_w, w_qkv, w_out, ln2_w, w_mlp_in, b_mlp_in,
               w_mlp_out, b_mlp_out):
    f32 = np.float32
    x = np.asarray(x, f32)
    x_flat = np.ascontiguousarray(x.reshape(NTOK, D))
    w_qkv = np.asarray(w_qkv, f32)
    w_out = np.asarray(w_out, f32)
    w_mlp_in = np.asarray(w_mlp_in, f32)
    w_mlp_out = np.asarray(w_mlp_out, f32)
    b_mlp_in = np.asarray(b_mlp_in, f32)
    b_mlp_out = np.asarray(b_mlp_out, f32)
    ln2_w = np.asarray(ln2_w, f32)
    alibi = np.asarray(alibi, f32)

    # x8 [p, pp, s, tok] = x[tok, (pp*2+s)*128 + p]
    x8 = np.ascontiguousarray(
        x_flat.T.reshape(4, 2, 128, NTOK).transpose(2, 0, 1, 3)).astype(E4M3)

    # identities
    i240 = (IDENT_V * np.eye(128, dtype=f32)).astype(E4M3)
    identb = np.eye(128, dtype=f32).astype(BF16)

    # out-proj weights: wo8[p, pp, s, dcol] = WS * w_out[dcol, (pp*2+s)*128+p]
    wo8 = np.ascontiguousarray(
        (WS * w_out).T.reshape(4, 2, 128, D).transpose(2, 0, 1, 3)
    ).astype(E4M3)

    w_in_eff = WS * w_mlp_in * ln2_w[None, :]     # (FF, D), x32 for fp8
    w_inP = np.ascontiguousarray(
        w_in_eff.reshape(32, 128, 8, 128).transpose(3, 0, 2, 1))
    w8i_h = w_inP.astype(E4M3)
    w8i_l = (w_inP - w8i_h.astype(np.float32)).astype(E4M3)
    # packed hi/lo: [p, ff, hl, kk, fin]
    w8i = np.ascontiguousarray(
        np.stack([w8i_h, w8i_l], axis=2))
    # w2P [ffp, p, s, dcol] = (32*w_mlp_out)[dcol, (ffp*2+s)*128+p]
    w2P = np.ascontiguousarray(
        (WS * w_mlp_out).T.reshape(16, 2, 128, D).transpose(0, 2, 1, 3))
    w8o_h = w2P.astype(E4M3)
    w8o_l = (w2P - w8o_h.astype(np.float32)).astype(E4M3)
    # packed hi/lo: [ffp, p, hl, s, dcol]
    w8o = np.ascontiguousarray(np.stack([w8o_h, w8o_l], axis=2))
    b_inT = np.ascontiguousarray(b_mlp_in.reshape(32, 128).T)

    in_maps = []
    for c in range(NCORES):
        h0 = HPC * c
        qrows = WS * w_qkv[h0 * Dh:(h0 + HPC) * Dh]
        krows = WS * w_qkv[H * Dh + h0 * Dh:H * Dh + (h0 + HPC) * Dh]
        vrows = WS * w_qkv[2 * H * Dh + h0 * Dh:2 * H * Dh + (h0 + HPC) * Dh]
        wq = np.concatenate([qrows, krows, vrows], 0)   # [384, D]
        # wq8 [p, pp, s, mcol] = wq[mcol, (pp*2+s)*128 + p]
        wq8 = np.ascontiguousarray(
            wq.T.reshape(4, 2, 128, 384).transpose(2, 0, 1, 3)).astype(E4M3)
        # alibi transposed to [hl, k, q]; k-row blocks gathered per engine
        # assignment (al8 row-block i = ACT_KTS[i], ab16 row-block i =
        # DVE_KTS[i])
        alc = alibi[0, h0:h0 + HPC].transpose(0, 2, 1)  # [HPC, T(k), T(q)]
        alc_kt = alc.reshape(HPC, KT, 128, T)
        al8 = np.ascontiguousarray(
            AL_SCALE * alc_kt[:, ACT_KTS].reshape(HPC, NAK * 128, T)
        ).astype(E4M3)
        ab16 = np.ascontiguousarray(
            LOG2E8 * alc_kt[:, DVE_KTS].reshape(HPC, NBK * 128, T) + BT_BIAS
        ).astype(np.float16)
        # residual pre-scaled x32: it is injected into the x32-scaled
        # out-proj / mlp-out PSUMs via f32r identity matmuls
        x_res = np.ascontiguousarray(WS * x_flat[c * CHUNK:(c + 1) * CHUNK])
        in_maps.append({
            "x8": x8, "wq8": wq8, "al8": al8, "ab16": ab16, "i240": i240,
            "identb": identb, "wo8": wo8,
            "x_res": x_res, "b_outT": b_mlp_out.reshape(1, D),
            "w8i": w8i, "b_inT": b_inT, "w8o": w8o,
        })
    return in_maps


def _get_compiled():
    global _COMPILED
    if _COMPILED is None:
        _COMPILED = _build()
    return _COMPILED


def kernel(_trace=False, **inputs):
    nc = _get_compiled()
    in_maps = _host_prep(**inputs)
    res = None
    for attempt in range(3):
        try:
            res = run_bass_kernel_spmd(nc, in_maps,
                                       core_ids=list(range(NCORES)),
                                       trace=_trace)
            break
        except Exception:
            if attempt == 2:
                raise
    out = np.concatenate([res.results[c]["out"] for c in range(NCORES)], 0)
    out = out.reshape(B, T, D).astype(np.float32)
    if _trace:
        return out, res
    return out
